# revision 20
# baseline (speedup 1.0000x reference)
"""Trainium2 Bass kernel for nn_BlockWithAdapter (B=2, T=2048, C=1024, H=16, M=64).

Strategy: token-parallel over 8 cores (core c owns 512 tokens of batch c//4).
Residual stream kept transposed [feature, token] on-chip so every linear layer
is a natural TensorE matmul (weights stationary, activations moving, N=512).
Causal attention computed dense per core (512 q x 2048 k, all 16 heads) with a
per-core multiplicative {0,1} mask applied after exp; softmax without
max-subtraction (scores are unit-scale by construction). K.T and V(plain) are
AllGathered within each batch group of 4 cores — the only collective.
All matmuls run as float32r (full fp32 data, full-rate PE mode at N>=256).
"""
import sys
import types

sys.path.insert(0, '/opt/trn_rl_repo')

import ml_dtypes
import numpy as np

import concourse.bass as bass  # noqa: F401  (registers arch)
import concourse.mybir as mybir
import concourse.tile as tile
from concourse import bacc
from concourse import bass_utils

P = 128
B, T, C, H, M = 2, 2048, 1024, 16, 64
HD = C // H            # 64
R = 512                # tokens per core
CT = C // P            # 8 feature tiles of the residual stream
NCORES = 8
RC = R * C             # floats in one K.T (or V) local buffer
EPS = 1e-5

FT = mybir.dt.float32
FR = mybir.dt.float32r
BF = mybir.dt.bfloat16
AF = mybir.ActivationFunctionType
OP = mybir.AluOpType

_CACHE = {}


def _fr(ap):
    return ap


def _build(stage='full'):
    nc = bacc.Bacc("TRN2", target_bir_lowering=False, debug=False,
                   num_devices=NCORES)

    # ---- DRAM I/O (per-core inputs; weights replicated across cores) ----
    d_xT = nc.dram_tensor("xT", [P, CT, R], FT, kind="ExternalInput")
    d_mask = nc.dram_tensor("mask", [P, 16, R], BF, kind="ExternalInput")
    d_wqk = nc.dram_tensor("wqk", [16, P, CT, P], BF, kind="ExternalInput")
    d_wv = nc.dram_tensor("wv", [2, P, CT, 512], BF, kind="ExternalInput")
    d_wproj = nc.dram_tensor("wproj", [CT, P, CT, P], BF, kind="ExternalInput")
    d_wfc = nc.dram_tensor("wfc", [32, P, CT, P], BF, kind="ExternalInput")
    d_wmp = nc.dram_tensor("wmp", [CT, P, 32, P], BF, kind="ExternalInput")
    d_a1d = nc.dram_tensor("a1d", [P, CT, M], BF, kind="ExternalInput")
    d_a1u = nc.dram_tensor("a1u", [M, CT, P], BF, kind="ExternalInput")
    d_a2d = nc.dram_tensor("a2d", [P, CT, M], BF, kind="ExternalInput")
    d_a2u = nc.dram_tensor("a2u", [M, CT, P], BF, kind="ExternalInput")
    d_bqk = nc.dram_tensor("bqk", [P, 16], FT, kind="ExternalInput")
    d_bv = nc.dram_tensor("bv", [2, 512], BF, kind="ExternalInput")
    d_bproj = nc.dram_tensor("bproj", [P, CT], FT, kind="ExternalInput")
    d_bfc = nc.dram_tensor("bfc", [P, 32], FT, kind="ExternalInput")
    d_bmp = nc.dram_tensor("bmp", [P, CT], FT, kind="ExternalInput")
    d_b1d = nc.dram_tensor("b1d", [M, 1], FT, kind="ExternalInput")
    d_b1u = nc.dram_tensor("b1u", [P, CT], FT, kind="ExternalInput")
    d_b2d = nc.dram_tensor("b2d", [M, 1], FT, kind="ExternalInput")
    d_b2u = nc.dram_tensor("b2u", [P, CT], FT, kind="ExternalInput")
    d_ln1g = nc.dram_tensor("ln1g", [P, CT], FT, kind="ExternalInput")
    d_ln1b = nc.dram_tensor("ln1b", [P, CT], FT, kind="ExternalInput")
    d_ln2g = nc.dram_tensor("ln2g", [P, CT], FT, kind="ExternalInput")
    d_ln2b = nc.dram_tensor("ln2b", [P, CT], FT, kind="ExternalInput")
    d_selc = nc.dram_tensor("selc", [2, 5 * P], BF, kind="ExternalInput")
    d_ones = nc.dram_tensor("onesc", [P, 1], BF, kind="ExternalInput")
    d_onesr = nc.dram_tensor("onesr", [1, P], BF, kind="ExternalInput")
    d_vones = nc.dram_tensor("vones", [P, 16], BF, kind="ExternalInput")
    d_sel16 = nc.dram_tensor("sel16", [CT, 16, P], BF, kind="ExternalInput")
    d_out = nc.dram_tensor("out", [CT, P, R], FT, kind="ExternalOutput")

    with tile.TileContext(nc) as tc, \
         nc.allow_low_precision(reason="float32r is 4-byte fp32-rounded"):
        with tc.tile_pool(name="const", bufs=1) as cp, \
             tc.tile_pool(name="resid", bufs=1) as rp, \
             tc.tile_pool(name="psum", bufs=2, space="PSUM") as pp, \
             tc.tile_pool(name="lnsb", bufs=4) as lp, \
             tc.tile_pool(name="sq", bufs=3) as sqp:

            # ---- constants ----
            ones = cp.tile([P, 1], BF, name="ones")
            nc.sync.dma_start(ones[:], d_ones[:])
            onesr = cp.tile([1, P], BF, name="onesr")
            nc.sync.dma_start(onesr[:], d_onesr[:])
            vones_sb = cp.tile([P, 16], BF, name="vones_sb")
            nc.sync.dma_start(vones_sb[:], d_vones[:])
            sel16_sb = cp.tile([16, CT, P], BF, name="sel16_sb")
            nc.sync.dma_start(sel16_sb[:], d_sel16[:].transpose([1, 0, 2]))
            onesf = cp.tile([P, 1], FT, name="onesf")
            nc.vector.memset(onesf[:], 1.0)
            eps_sb = cp.tile([1, 1], FT, name="eps_sb")
            nc.vector.memset(eps_sb[:], EPS)
            # selector matrices for broadcasting a row vector across
            # partitions via a K=2 matmul: out[m, :] = rhs[row(m), :]
            # (host-staged; memset cannot write partition base 1)
            selc_sb = cp.tile([2, 5 * P], BF, name="selc_sb")
            nc.sync.dma_start(selc_sb[:], d_selc[:])
            sel_a = selc_sb[:, 0:P]          # row 0 -> all 128 partitions
            sel_b = selc_sb[:, P:2 * P]      # row 1 -> all 128 partitions
            sel_h = selc_sb[:, 2 * P:3 * P]  # row m//64
            sel_lo = selc_sb[0:1, 3 * P:4 * P]   # row 0 -> partitions 0:64
            sel_hi = selc_sb[0:1, 4 * P:5 * P]   # row 0 -> partitions 64:128
            bqk_sb = cp.tile([P, 16], FT, name="bqk_sb")
            nc.sync.dma_start(bqk_sb[:], d_bqk[:])
            bv_sb = cp.tile([2, 512], BF, name="bv_sb")
            nc.sync.dma_start(bv_sb[:], d_bv[:])
            bproj_sb = cp.tile([P, CT], FT, name="bproj_sb")
            nc.sync.dma_start(bproj_sb[:], d_bproj[:])
            bfc_sb = cp.tile([P, 32], FT, name="bfc_sb")
            nc.sync.dma_start(bfc_sb[:], d_bfc[:])
            bmp_sb = cp.tile([P, CT], FT, name="bmp_sb")
            nc.sync.dma_start(bmp_sb[:], d_bmp[:])
            b1d_sb = cp.tile([M, 1], FT, name="b1d_sb")
            nc.sync.dma_start(b1d_sb[:], d_b1d[:])
            b1u_sb = cp.tile([P, CT], FT, name="b1u_sb")
            nc.sync.dma_start(b1u_sb[:], d_b1u[:])
            b2d_sb = cp.tile([M, 1], FT, name="b2d_sb")
            nc.sync.dma_start(b2d_sb[:], d_b2d[:])
            b2u_sb = cp.tile([P, CT], FT, name="b2u_sb")
            nc.sync.dma_start(b2u_sb[:], d_b2u[:])
            ln1g_sb = cp.tile([P, CT], FT, name="ln1g_sb")
            nc.sync.dma_start(ln1g_sb[:], d_ln1g[:])
            ln1b_sb = cp.tile([P, CT], FT, name="ln1b_sb")
            nc.sync.dma_start(ln1b_sb[:], d_ln1b[:])
            ln2g_sb = cp.tile([P, CT], FT, name="ln2g_sb")
            nc.sync.dma_start(ln2g_sb[:], d_ln2g[:])
            ln2b_sb = cp.tile([P, CT], FT, name="ln2b_sb")
            nc.sync.dma_start(ln2b_sb[:], d_ln2b[:])
            # causal mask resident in SBUF (was re-streamed per (hp, kb):
            # 64 MB of HBM traffic that throttled the attention phase).
            # Allocated here; DMA'd after the K-half-1 loop so startup DMA
            # bandwidth goes to x and the first weight tiles.
            msk_sb = cp.tile([P, 16, R], BF, name="msk_sb")
            # Q weight tiles preloaded so the Q matmuls run with no DMA
            # dependency during the K/V AllGather window
            wq_all = cp.tile([P, CT, CT, P], BF, name="wq_all")

            # ---- residual stream (transposed), loaded once ----
            X = []
            for ct in range(CT):
                xt = rp.tile([P, R], FT, name=f"x{ct}")
                nc.sync.dma_start(xt[:], d_xT[:, ct, :])
                X.append(xt)

            STAGES = ['x', 'ln1', 'qkv', 'attn', 'proj', 'a1', 'mlp', 'full']
            s_idx = STAGES.index(stage)

            def want(s):
                return s_idx >= STAGES.index(s)

            def dump(tiles):
                for i, t in enumerate(tiles[:CT]):
                    if t[:].dtype == FT:
                        nc.sync.dma_start(d_out[i], t[:])
                    else:
                        nc.gpsimd.dma_start(d_out[i], t[:])

            def layer_norm(dst_pool, g_sb, b_sb, tag):
                """LN over the feature axis of the transposed stream."""
                s_ps = pp.tile([1, R], FT, name=f"s_{tag}", tag="mm")
                q_ps = pp.tile([1, R], FT, name=f"q_{tag}", tag="mm")
                xb = []
                for ct in range(CT):
                    xbt = sqp.tile([P, R], BF, name=f"xb_{tag}", tag="xb",
                                   bufs=CT)
                    nc.vector.tensor_copy(xbt[:], X[ct][:])
                    xb.append(xbt)
                    xsq = sqp.tile([P, R], BF, name=f"xsq_{tag}", tag="xsq")
                    nc.scalar.activation(xsq[:], X[ct][:], AF.Square)
                    nc.tensor.matmul(s_ps[:], _fr(ones[:]), _fr(xbt[:]),
                                     start=(ct == 0), stop=(ct == CT - 1))
                    nc.tensor.matmul(q_ps[:], _fr(ones[:]), _fr(xsq[:]),
                                     start=(ct == 0), stop=(ct == CT - 1))
                mu = lp.tile([1, R], BF, name=f"mu_{tag}", tag="ln")
                nc.scalar.mul(mu[:], s_ps[:], 1.0 / C)
                ex2 = lp.tile([1, R], FT, name=f"ex2_{tag}", tag="ln")
                nc.scalar.mul(ex2[:], q_ps[:], 1.0 / C)
                var = lp.tile([1, R], FT, name=f"var_{tag}", tag="ln")
                nc.vector.tensor_mul(var[:], mu[:], mu[:])
                nc.vector.tensor_sub(var[:], ex2[:], var[:])
                std = lp.tile([1, R], FT, name=f"std_{tag}", tag="ln")
                nc.scalar.activation(std[:], var[:], AF.Sqrt, bias=eps_sb[:])
                rstd = lp.tile([1, R], BF, name=f"rstd_{tag}", tag="ln")
                nc.vector.reciprocal(rstd[:], std[:])
                # broadcast mu and rstd across all 128 partitions (into PSUM)
                mu_b = pp.tile([P, R], FT, name=f"mub_{tag}", tag="mm")
                nc.tensor.matmul(mu_b[:], _fr(onesr[:]), _fr(mu[:]))
                rstd_b = pp.tile([P, R], FT, name=f"rsb_{tag}", tag="mm")
                nc.tensor.matmul(rstd_b[:], _fr(onesr[:]), _fr(rstd[:]))
                out_tiles = []
                for ct in range(CT):
                    xn = dst_pool.tile([P, R], BF, name=f"{tag}_{ct}", tag="xln")
                    nc.vector.tensor_sub(xn[:], X[ct][:], mu_b[:])
                    nc.vector.tensor_mul(xn[:], xn[:], rstd_b[:])
                    nc.scalar.activation(xn[:], xn[:], AF.Identity,
                                         scale=g_sb[:, ct:ct + 1],
                                         bias=b_sb[:, ct:ct + 1])
                    out_tiles.append(xn)
                return out_tiles

            with tc.tile_pool(name="qT", bufs=1) as qp, \
                 tc.tile_pool(name="oT", bufs=1) as op, \
                 tc.tile_pool(name="dram", bufs=1, space="DRAM") as dp:

                # K/V exchanged in four half-sized AllGathers (K heads 0-7,
                # V heads 0-7, K heads 8-15, V heads 8-15), each triggered as
                # soon as its half is computed so the slow ncfw collectives
                # pipeline under the remaining QKV matmuls and the first
                # attention head-pairs.
                HC = RC // 2
                cc_ki = [dp.tile([HC], BF, name=f"cc_k{i}i") for i in range(2)]
                cc_ko = [dp.tile([4, HC], BF, name=f"cc_k{i}o") for i in range(2)]
                cc_vi = [dp.tile([HC], BF, name=f"cc_v{i}i") for i in range(2)]
                cc_vo = [dp.tile([4, HC], BF, name=f"cc_v{i}o") for i in range(2)]
                ccin_k = [t[:].rearrange("(f t) -> f t", f=C // 2)
                          for t in cc_ki]
                ccin_v = [t[:].rearrange("(t f) -> t f", t=R) for t in cc_vi]
                # dummy collective fired immediately at core start: the
                # implicit pre-collective barrier then absorbs the multi-core
                # launch skew while startup DMAs run, instead of stalling the
                # first real gather
                cc_di = dp.tile([256], BF, name="cc_di")
                cc_do = dp.tile([8, 256], BF, name="cc_do")
                if want('qkv'):
                    nc.gpsimd.collective_compute(
                        "AllGather", OP.bypass,
                        replica_groups=[[0, 1, 2, 3, 4, 5, 6, 7]],
                        ins=[cc_di[:]], outs=[cc_do[:]])

                # ================= LN1 + QKV + AllGather =================
                with tc.tile_pool(name="xln1", bufs=CT) as x1p, \
                     tc.tile_pool(name="wq", bufs=3) as wqp, \
                     tc.tile_pool(name="wvp", bufs=2) as wvp, \
                     tc.tile_pool(name="kvb", bufs=4) as kvbp:

                    xln = (layer_norm(x1p, ln1g_sb, ln1b_sb, "ln1")
                           if want('ln1') else [])
                    if stage == 'ln1':
                        dump(xln)

                    def k_half(h):
                        # K features [512h, 512h+512) = heads [8h, 8h+8)
                        for ft in range(4 * h, 4 * h + 4):
                            mt = 8 + ft
                            wt = wqp.tile([P, CT, P], BF, name="wqk_t",
                                          tag="wqk")
                            nc.sync.dma_start(wt[:], d_wqk[mt])
                            ps = pp.tile([P, R], FT, name="ps_k", tag="mm")
                            for kt in range(CT):
                                nc.tensor.matmul(ps[:], _fr(wt[:, kt, :]),
                                                 _fr(xln[kt][:]),
                                                 start=(kt == 0),
                                                 stop=(kt == CT - 1))
                            kt_sb = kvbp.tile([P, R], BF, name="kt_sb",
                                              tag="kvb")
                            nc.scalar.activation(kt_sb[:], ps[:], AF.Identity,
                                                 bias=bqk_sb[:, mt:mt + 1])
                            lf = ft - 4 * h
                            nc.sync.dma_start(
                                ccin_k[h][P * lf:P * (lf + 1), :], kt_sb[:])

                    def v_half(nt):
                        wv_sb = wvp.tile([P, CT, 512], BF, name="wv_sb",
                                         tag="wv")
                        nc.sync.dma_start(wv_sb[:], d_wv[nt])
                        sel_v = sel_a if nt == 0 else sel_b
                        for mt in range(4):
                            ps = pp.tile([P, 512], FT, name="ps_v", tag="mm")
                            for kt in range(CT):
                                nc.tensor.matmul(
                                    ps[:],
                                    _fr(xln[kt][:, P * mt:P * (mt + 1)]),
                                    _fr(wv_sb[:, kt, :]),
                                    start=(kt == 0), stop=False)
                            # fold the (free-axis) V bias in as a rank-2
                            # broadcast matmul accumulated into the same bank
                            nc.tensor.matmul(ps[:], _fr(sel_v[:]),
                                             _fr(bv_sb[:]),
                                             start=False, stop=True)
                            v_sb = kvbp.tile([P, 512], BF, name="v_sb",
                                             tag="kvb")
                            nc.scalar.copy(v_sb[:], ps[:])
                            nc.sync.dma_start(
                                ccin_v[nt][P * mt:P * (mt + 1), :], v_sb[:])

                    def gather(ins, outs):
                        nc.gpsimd.collective_compute(
                            "AllGather", OP.bypass,
                            replica_groups=[[0, 1, 2, 3], [4, 5, 6, 7]],
                            ins=[ins[:]], outs=[outs[:]])

                    if want('qkv'):
                        k_half(0)
                        gather(cc_ki[0], cc_ko[0])
                        # deferred preloads: mask + Q weights land while the
                        # first gather is in flight
                        nc.sync.dma_start(msk_sb[:], d_mask[:])
                        nc.sync.dma_start(wq_all[:],
                                          d_wqk[0:8].transpose([1, 0, 2, 3]))
                        v_half(0)
                        gather(cc_vi[0], cc_vo[0])
                        k_half(1)
                        gather(cc_ki[1], cc_ko[1])
                        v_half(1)
                        gather(cc_vi[1], cc_vo[1])

                    # Q tiles (pre-scaled by 1/8 host-side), stay resident.
                    # Weights were preloaded, so these matmuls run during the
                    # AllGather with zero DMA dependencies.
                    qT = []
                    for mt in (range(CT) if want('qkv') else []):
                        ps = pp.tile([P, R], FT, name="ps_q", tag="mm")
                        for kt in range(CT):
                            nc.tensor.matmul(ps[:], _fr(wq_all[:, mt, kt, :]),
                                             _fr(xln[kt][:]),
                                             start=(kt == 0), stop=(kt == CT - 1))
                        qt = qp.tile([P, R], BF, name=f"q{mt}")
                        nc.scalar.activation(qt[:], ps[:], AF.Identity,
                                             bias=bqk_sb[:, mt:mt + 1])
                        qT.append(qt)
                    if stage == 'qkv':
                        dump(qT)

                # ======================= attention =======================
                with tc.tile_pool(name="kv", bufs=3) as kvp, \
                     tc.tile_pool(name="pexp", bufs=8) as pep, \
                     tc.tile_pool(name="maskp", bufs=6) as mp, \
                     tc.tile_pool(name="wp", bufs=1) as wpp:

                    # prefetch proj weights on the idle GpSimd DMA queue so
                    # they land during attention (the Sync queue stays free
                    # for the per-head K/V loads)
                    wp_all = wpp.tile([P, CT, CT, P], BF, name="wp_all")
                    if want('proj'):
                        nc.gpsimd.dma_start(
                            wp_all[:], d_wproj[:].transpose([1, 0, 2, 3]))
                    oT = []
                    for hp in (range(CT) if want('attn') else []):
                        kT_h = kvp.tile([P, 16, P], BF, name="kT_h", tag="kT")
                        v_h = kvp.tile([P, 16, 130], BF, name="v_h", tag="vT")
                        # ones columns (64 and 129) for in-matmul denominators
                        nc.sync.dma_start(v_h[:, :, 64:65],
                                          vones_sb[:].unsqueeze(2))
                        nc.sync.dma_start(v_h[:, :, 129:130],
                                          vones_sb[:].unsqueeze(2))
                        half, lp_ = hp // 4, hp % 4
                        for r in range(4):
                            src_k = cc_ko[half][r].rearrange(
                                "(f t) -> f t", f=C // 2)[
                                P * lp_:P * (lp_ + 1), :]
                            nc.sync.dma_start(
                                kT_h[:, 4 * r:4 * (r + 1), :],
                                src_k.rearrange("p (tb t) -> p tb t", tb=4))
                            src_v = cc_vo[half][r].rearrange(
                                "(tb p f) -> p tb f", tb=4, p=P)[
                                :, :, P * lp_:P * (lp_ + 1)]
                            nc.sync.dma_start(
                                v_h[:, 4 * r:4 * (r + 1), 0:64],
                                src_v[:, :, 0:64])
                            nc.sync.dma_start(
                                v_h[:, 4 * r:4 * (r + 1), 65:129],
                                src_v[:, :, 64:P])

                        o_ps0 = pp.tile([65, R], FT, name="o_ps0", tag="opsum",
                                        bufs=2)
                        o_ps1 = pp.tile([65, R], FT, name="o_ps1", tag="opsum",
                                        bufs=2)
                        for kb in range(16):
                            s0 = pp.tile([P, 2 * R], FT, name="s0", tag="spsum",
                                         bufs=2)
                            nc.tensor.matmul(s0[:, 0:R], _fr(kT_h[0:64, kb, :]),
                                             _fr(qT[hp][0:64, :]))
                            nc.tensor.matmul(s0[:, R:2 * R],
                                             _fr(kT_h[64:P, kb, :]),
                                             _fr(qT[hp][64:P, :]))
                            p0 = pep.tile([P, 2 * R], BF, name="p0", tag="pexp")
                            nc.scalar.activation(p0[:], s0[:], AF.Exp)
                            nc.vector.tensor_mul(p0[:, 0:R], p0[:, 0:R],
                                                 msk_sb[:, kb, :])
                            nc.vector.tensor_mul(p0[:, R:2 * R], p0[:, R:2 * R],
                                                 msk_sb[:, kb, :])
                            nc.tensor.matmul(o_ps0[:],
                                             _fr(v_h[:, kb, 0:65]),
                                             _fr(p0[:, 0:R]),
                                             start=(kb == 0), stop=(kb == 15))
                            nc.tensor.matmul(o_ps1[:],
                                             _fr(v_h[:, kb, 65:130]),
                                             _fr(p0[:, R:2 * R]),
                                             start=(kb == 0), stop=(kb == 15))

                        # denominators ride along as psum row 64; normalize
                        # per head-pair so nothing serializes at the end
                        dt0 = mp.tile([1, R], FT, name="dt0", tag="dt", bufs=4)
                        dt1 = mp.tile([1, R], FT, name="dt1", tag="dt", bufs=4)
                        nc.scalar.copy(dt0[:], o_ps0[64:65, :])
                        nc.scalar.copy(dt1[:], o_ps1[64:65, :])
                        rd0 = mp.tile([1, R], BF, name="rd0", tag="rd", bufs=4)
                        rd1 = mp.tile([1, R], BF, name="rd1", tag="rd", bufs=4)
                        nc.vector.reciprocal(rd0[:], dt0[:])
                        nc.vector.reciprocal(rd1[:], dt1[:])
                        rdb = pp.tile([P, R], FT, name="rdb", tag="mm")
                        nc.tensor.matmul(rdb[:], _fr(sel_lo[:]), _fr(rd0[:]),
                                         start=True, stop=False)
                        nc.tensor.matmul(rdb[:], _fr(sel_hi[:]), _fr(rd1[:]),
                                         start=False, stop=True)
                        ot = op.tile([P, R], BF, name=f"o{hp}")
                        nc.vector.tensor_copy(ot[0:64, :], o_ps0[0:64, :])
                        nc.vector.tensor_copy(ot[64:P, :], o_ps1[0:64, :])
                        nc.vector.tensor_mul(ot[:], ot[:], rdb[:])
                        oT.append(ot)

                    if stage == 'attn':
                        dump(oT)

                    # ================ proj + residual ================
                    for mt in (range(CT) if want('proj') else []):
                        ps = pp.tile([P, R], FT, name="ps_pr", tag="mm")
                        for kt in range(CT):
                            nc.tensor.matmul(ps[:], _fr(wp_all[:, mt, kt, :]),
                                             _fr(oT[kt][:]),
                                             start=(kt == 0), stop=(kt == CT - 1))
                        nc.vector.scalar_tensor_tensor(
                            X[mt][:], ps[:], bproj_sb[:, mt:mt + 1], X[mt][:],
                            op0=OP.add, op1=OP.add)

            # ================== adapters + MLP ==================
            def adapter(d_dw, d_uw, bd_sb, bu_sb, tag):
                with tc.tile_pool(name=f"ad_{tag}", bufs=1) as adp:
                    ad = adp.tile([P, CT, M], BF, name=f"ad_{tag}")
                    nc.sync.dma_start(ad[:], d_dw[:])
                    au = adp.tile([M, CT, P], BF, name=f"au_{tag}")
                    nc.sync.dma_start(au[:], d_uw[:])
                    ps_a = pp.tile([M, R], FT, name=f"psa_{tag}", tag="mm")
                    for kt in range(CT):
                        xbt = adp.tile([P, R], BF, name=f"xb_{tag}", tag="xb",
                                       bufs=3)
                        nc.vector.tensor_copy(xbt[:], X[kt][:])
                        nc.tensor.matmul(ps_a[:], _fr(ad[:, kt, :]),
                                         _fr(xbt[:]),
                                         start=(kt == 0), stop=(kt == CT - 1))
                    ar = adp.tile([M, R], BF, name=f"ar_{tag}")
                    nc.scalar.activation(ar[:], ps_a[:], AF.Relu,
                                         bias=bd_sb[:, 0:1])
                    for mt in range(CT):
                        ps = pp.tile([P, R], FT, name=f"psu_{tag}", tag="mm")
                        nc.tensor.matmul(ps[:], _fr(au[:, mt, :]), _fr(ar[:]))
                        nc.vector.scalar_tensor_tensor(
                            X[mt][:], ps[:], bu_sb[:, mt:mt + 1], X[mt][:],
                            op0=OP.add, op1=OP.add)

            if want('a1'):
                adapter(d_a1d, d_a1u, b1d_sb, b1u_sb, "a1")

            # ---------------- LN2 + MLP ----------------
            with tc.tile_pool(name="xln2", bufs=CT) as x2p, \
                 tc.tile_pool(name="wfcp", bufs=3) as wfp, \
                 tc.tile_pool(name="hT", bufs=32) as hp_, \
                 tc.tile_pool(name="wmpp", bufs=2) as wmp:

                x2 = layer_norm(x2p, ln2g_sb, ln2b_sb, "ln2") if want('mlp') else []
                hT = []
                for mt in (range(32) if want('mlp') else []):
                    wt = wfp.tile([P, CT, P], BF, name="wfc_t", tag="wfc")
                    nc.sync.dma_start(wt[:], d_wfc[mt])
                    ps = pp.tile([P, R], FT, name="ps_fc", tag="mm")
                    for kt in range(CT):
                        nc.tensor.matmul(ps[:], _fr(wt[:, kt, :]), _fr(x2[kt][:]),
                                         start=(kt == 0), stop=(kt == CT - 1))
                    ht = hp_.tile([P, R], BF, name="ht", tag="hT")
                    nc.scalar.activation(ht[:], ps[:], AF.Gelu_apprx_tanh,
                                         bias=bfc_sb[:, mt:mt + 1])
                    hT.append(ht)
                for mt in (range(CT) if want('mlp') else []):
                    wt = wmp.tile([P, 32, P], BF, name="wmp_t", tag="wmp")
                    nc.sync.dma_start(wt[:], d_wmp[mt])
                    ps = pp.tile([P, R], FT, name="ps_mp", tag="mm")
                    for kt in range(32):
                        nc.tensor.matmul(ps[:], _fr(wt[:, kt, :]), _fr(hT[kt][:]),
                                         start=(kt == 0), stop=(kt == 31))
                    nc.vector.scalar_tensor_tensor(
                        X[mt][:], ps[:], bmp_sb[:, mt:mt + 1], X[mt][:],
                        op0=OP.add, op1=OP.add)

            if want('full'):
                adapter(d_a2d, d_a2u, b2d_sb, b2u_sb, "a2")

            # ---------------- output ----------------
            if stage in ('x', 'proj', 'a1', 'mlp', 'full'):
                for ct in range(CT):
                    nc.sync.dma_start(d_out[ct], X[ct][:])

    nc.compile()
    return nc


def _prep_shared(inputs):
    """Host-side tiling of weights/biases into the DRAM layouts above."""
    f32 = np.float32
    bf16 = ml_dtypes.bfloat16
    W = {k: np.ascontiguousarray(np.asarray(v, dtype=f32))
         for k, v in inputs.items()}
    s = f32(1.0 / np.sqrt(HD))
    aw = W['attn_w']
    ab = W['attn_b']
    wq = aw[:, :C] * s
    wk = aw[:, C:2 * C]
    wv = aw[:, 2 * C:]

    def lhst_tiles(w, nmt):
        # w [K, Mout] -> [nmt, P, K//P, P]: tile[mt, p, kt, m] = w[P*kt+p, P*mt+m]
        kk, mm = w.shape
        return np.ascontiguousarray(
            w.reshape(kk // P, P, nmt, P).transpose(2, 1, 0, 3))

    wqk = np.concatenate([lhst_tiles(wq, 8), lhst_tiles(wk, 8)], axis=0)
    # wv moving tiles [2, P, CT, 512]: tile[nt, p, kt, m] = wv[P*kt+p, 512*nt+m]
    wv_m = np.ascontiguousarray(
        wv.reshape(CT, P, 2, 512).transpose(2, 1, 0, 3))

    def col_vec(v, nmt):
        # v [nmt*P] -> [P, nmt]
        return np.ascontiguousarray(v.reshape(nmt, P).T)

    shared = {
        'wqk': wqk,
        'wv': wv_m,
        'wproj': lhst_tiles(W['proj_w'], CT),
        'wfc': lhst_tiles(W['fc_w'], 32),
        'wmp': lhst_tiles(W['mlp_pw'], CT),
        'a1d': np.ascontiguousarray(
            W['a1_dw'].reshape(CT, P, M).transpose(1, 0, 2)),
        'a1u': np.ascontiguousarray(W['a1_uw'].reshape(M, CT, P)),
        'a2d': np.ascontiguousarray(
            W['a2_dw'].reshape(CT, P, M).transpose(1, 0, 2)),
        'a2u': np.ascontiguousarray(W['a2_uw'].reshape(M, CT, P)),
        'bqk': np.ascontiguousarray(
            np.concatenate([ab[:C] * s, ab[C:2 * C]]).reshape(16, P).T),
        'bv': np.ascontiguousarray(ab[2 * C:].reshape(2, 512)),
        'bproj': col_vec(W['proj_b'], CT),
        'bfc': col_vec(W['fc_b'], 32),
        'bmp': col_vec(W['mlp_pb'], CT),
        'b1d': np.ascontiguousarray(W['a1_db'].reshape(M, 1)),
        'b1u': col_vec(W['a1_ub'], CT),
        'b2d': np.ascontiguousarray(W['a2_db'].reshape(M, 1)),
        'b2u': col_vec(W['a2_ub'], CT),
        'ln1g': col_vec(W['ln1_g'], CT),
        'ln1b': col_vec(W['ln1_b'], CT),
        'ln2g': col_vec(W['ln2_g'], CT),
        'ln2b': col_vec(W['ln2_b'], CT),
    }
    selc = np.zeros((2, 5 * P), dtype=f32)
    selc[0, 0:P] = 1.0                       # sel_a: row 0 everywhere
    selc[1, P:2 * P] = 1.0                   # sel_b: row 1 everywhere
    selc[0, 2 * P:2 * P + 64] = 1.0          # sel_h: row m//64
    selc[1, 2 * P + 64:3 * P] = 1.0
    selc[0, 3 * P:3 * P + 64] = 1.0          # sel_lo: partitions 0:64
    selc[0, 4 * P + 64:5 * P] = 1.0          # sel_hi: partitions 64:128
    shared['selc'] = selc
    shared['onesc'] = np.ones((P, 1), dtype=f32)
    shared['onesr'] = np.ones((1, P), dtype=f32)
    shared['vones'] = np.ones((P, 16), dtype=f32)
    sel16 = np.zeros((CT, 16, P), dtype=f32)
    for hp in range(CT):
        sel16[hp, 2 * hp, 0:64] = 1.0
        sel16[hp, 2 * hp + 1, 64:P] = 1.0
    shared['sel16'] = sel16
    for k in ('wqk', 'wv', 'wproj', 'wfc', 'wmp', 'a1d', 'a1u', 'a2d', 'a2u',
              'bv', 'selc', 'onesc', 'onesr', 'vones', 'sel16'):
        shared[k] = np.ascontiguousarray(shared[k].astype(bf16))
    return shared


def _prep_core(x, c):
    b, lc = c // 4, c % 4
    xl = np.asarray(x[b, R * lc:R * (lc + 1), :], dtype=np.float32)   # [R, C]
    xT = np.ascontiguousarray(
        xl.T.reshape(CT, P, R).transpose(1, 0, 2))                    # [P, CT, R]
    kj = np.arange(T)[:, None]
    qi = np.arange(R)[None, :] + R * lc
    mask = (kj <= qi).astype(np.float32)                              # [T, R]
    m3 = mask.reshape(16, P, R).transpose(1, 0, 2)                    # [P, 16, R]
    maskT = np.ascontiguousarray(m3.astype(ml_dtypes.bfloat16))
    return {'xT': xT, 'mask': maskT}


def _run(inputs, trace=False, stage='full'):
    if stage not in _CACHE:
        _CACHE[stage] = _build(stage)
    nc = _CACHE[stage]
    shared = _prep_shared(inputs)
    x = np.asarray(inputs['x'], dtype=np.float32)
    in_maps = []
    for c in range(NCORES):
        m = dict(shared)
        m.update(_prep_core(x, c))
        in_maps.append(m)
    kwargs = {}
    if trace:
        from trn_agent_boot.trn_boot import _ntff_profile_via_ctypes
        hook = _ntff_profile_via_ctypes('/opt/axon/libaxon_pjrt.so')
        mod = types.ModuleType('antenv.axon_hooks')
        mod.get_axon_ntff_profile_hook = lambda: hook
        sys.modules['antenv.axon_hooks'] = mod
        bass_utils.upload_artifacts = lambda tmpdir: "/tmp/no-upload"
        kwargs['trace'] = True
    res = bass_utils.run_bass_kernel_spmd(
        nc, in_maps, core_ids=list(range(NCORES)), **kwargs)
    y = np.zeros((B, T, C), dtype=np.float32)
    for c in range(NCORES):
        b, lc = c // 4, c % 4
        o = res.results[c]['out']          # [CT, P, R]
        y[b, R * lc:R * (lc + 1), :] = o.reshape(C, R).T
    return y, res


def kernel(**inputs):
    y, _ = _run(inputs, trace=False)
    return y



# revision 25
# speedup vs baseline: 1.0853x; 1.0853x over previous
"""Trainium2 Bass kernel for nn_BlockWithAdapter (B=2, T=2048, C=1024, H=16, M=64).

Strategy: token-parallel over 8 cores (core c owns 512 tokens of batch c//4).
Residual stream kept transposed [feature, token] on-chip so every linear layer
is a natural TensorE matmul (weights stationary, activations moving, N=512).
Causal attention computed dense per core (512 q x 2048 k, all 16 heads) with a
per-core multiplicative {0,1} mask applied after exp; softmax without
max-subtraction (scores are unit-scale by construction). K.T and V(plain) are
AllGathered within each batch group of 4 cores — the only collective.
All matmuls run as float32r (full fp32 data, full-rate PE mode at N>=256).
"""
import sys
import types

sys.path.insert(0, '/opt/trn_rl_repo')

import ml_dtypes
import numpy as np

import concourse.bass as bass  # noqa: F401  (registers arch)
import concourse.mybir as mybir
import concourse.tile as tile
from concourse import bacc
from concourse import bass_utils

P = 128
B, T, C, H, M = 2, 2048, 1024, 16, 64
HD = C // H            # 64
R = 512                # tokens per core
CT = C // P            # 8 feature tiles of the residual stream
NCORES = 8
RC = R * C             # floats in one K.T (or V) local buffer
EPS = 1e-5

FT = mybir.dt.float32
FR = mybir.dt.float32r
BF = mybir.dt.bfloat16
AF = mybir.ActivationFunctionType
OP = mybir.AluOpType

_CACHE = {}


def _fr(ap):
    return ap


def _build(stage='full'):
    nc = bacc.Bacc("TRN2", target_bir_lowering=False, debug=False,
                   num_devices=NCORES)

    # ---- DRAM I/O (per-core inputs; weights replicated across cores) ----
    d_xT = nc.dram_tensor("xT", [P, CT, R], FT, kind="ExternalInput")
    d_mask = nc.dram_tensor("mask", [P, 16, R], BF, kind="ExternalInput")
    d_wqk = nc.dram_tensor("wqk", [16, P, CT, P], BF, kind="ExternalInput")
    d_wv = nc.dram_tensor("wv", [2, P, CT, 512], BF, kind="ExternalInput")
    d_wproj = nc.dram_tensor("wproj", [CT, P, CT, P], BF, kind="ExternalInput")
    d_wfc = nc.dram_tensor("wfc", [32, P, CT, P], BF, kind="ExternalInput")
    d_wmp = nc.dram_tensor("wmp", [CT, P, 32, P], BF, kind="ExternalInput")
    d_a1d = nc.dram_tensor("a1d", [P, CT, M], BF, kind="ExternalInput")
    d_a1u = nc.dram_tensor("a1u", [M, CT, P], BF, kind="ExternalInput")
    d_a2d = nc.dram_tensor("a2d", [P, CT, M], BF, kind="ExternalInput")
    d_a2u = nc.dram_tensor("a2u", [M, CT, P], BF, kind="ExternalInput")
    d_bqk = nc.dram_tensor("bqk", [P, 16], FT, kind="ExternalInput")
    d_bv = nc.dram_tensor("bv", [2, 512], BF, kind="ExternalInput")
    d_bproj = nc.dram_tensor("bproj", [P, CT], FT, kind="ExternalInput")
    d_bfc = nc.dram_tensor("bfc", [P, 32], FT, kind="ExternalInput")
    d_bmp = nc.dram_tensor("bmp", [P, CT], FT, kind="ExternalInput")
    d_b1d = nc.dram_tensor("b1d", [M, 1], FT, kind="ExternalInput")
    d_b1u = nc.dram_tensor("b1u", [P, CT], FT, kind="ExternalInput")
    d_b2d = nc.dram_tensor("b2d", [M, 1], FT, kind="ExternalInput")
    d_b2u = nc.dram_tensor("b2u", [P, CT], FT, kind="ExternalInput")
    d_ln1g = nc.dram_tensor("ln1g", [P, CT], FT, kind="ExternalInput")
    d_ln1b = nc.dram_tensor("ln1b", [P, CT], FT, kind="ExternalInput")
    d_ln2g = nc.dram_tensor("ln2g", [P, CT], FT, kind="ExternalInput")
    d_ln2b = nc.dram_tensor("ln2b", [P, CT], FT, kind="ExternalInput")
    d_selc = nc.dram_tensor("selc", [2, 5 * P], BF, kind="ExternalInput")
    d_ones = nc.dram_tensor("onesc", [P, 1], BF, kind="ExternalInput")
    d_onesr = nc.dram_tensor("onesr", [1, P], BF, kind="ExternalInput")
    d_vones = nc.dram_tensor("vones", [P, 16], BF, kind="ExternalInput")
    d_sel16 = nc.dram_tensor("sel16", [CT, 16, P], BF, kind="ExternalInput")
    d_out = nc.dram_tensor("out", [CT, P, R], FT, kind="ExternalOutput")

    with tile.TileContext(nc) as tc, \
         nc.allow_low_precision(reason="float32r is 4-byte fp32-rounded"):
        with tc.tile_pool(name="const", bufs=1) as cp, \
             tc.tile_pool(name="resid", bufs=1) as rp, \
             tc.tile_pool(name="psum", bufs=2, space="PSUM") as pp, \
             tc.tile_pool(name="lnsb", bufs=4) as lp, \
             tc.tile_pool(name="sq", bufs=3) as sqp:

            # ---- constants ----
            ones = cp.tile([P, 1], BF, name="ones")
            nc.sync.dma_start(ones[:], d_ones[:])
            onesr = cp.tile([1, P], BF, name="onesr")
            nc.sync.dma_start(onesr[:], d_onesr[:])
            vones_sb = cp.tile([P, 16], BF, name="vones_sb")
            nc.sync.dma_start(vones_sb[:], d_vones[:])
            sel16_sb = cp.tile([16, CT, P], BF, name="sel16_sb")
            nc.sync.dma_start(sel16_sb[:], d_sel16[:].transpose([1, 0, 2]))
            onesf = cp.tile([P, 1], FT, name="onesf")
            nc.vector.memset(onesf[:], 1.0)
            eps_sb = cp.tile([1, 1], FT, name="eps_sb")
            nc.vector.memset(eps_sb[:], EPS)
            # selector matrices for broadcasting a row vector across
            # partitions via a K=2 matmul: out[m, :] = rhs[row(m), :]
            # (host-staged; memset cannot write partition base 1)
            selc_sb = cp.tile([2, 5 * P], BF, name="selc_sb")
            nc.sync.dma_start(selc_sb[:], d_selc[:])
            sel_a = selc_sb[:, 0:P]          # row 0 -> all 128 partitions
            sel_b = selc_sb[:, P:2 * P]      # row 1 -> all 128 partitions
            sel_h = selc_sb[:, 2 * P:3 * P]  # row m//64
            sel_lo = selc_sb[0:1, 3 * P:4 * P]   # row 0 -> partitions 0:64
            sel_hi = selc_sb[0:1, 4 * P:5 * P]   # row 0 -> partitions 64:128
            bqk_sb = cp.tile([P, 16], FT, name="bqk_sb")
            nc.sync.dma_start(bqk_sb[:], d_bqk[:])
            bv_sb = cp.tile([2, 512], BF, name="bv_sb")
            nc.sync.dma_start(bv_sb[:], d_bv[:])
            bproj_sb = cp.tile([P, CT], FT, name="bproj_sb")
            nc.sync.dma_start(bproj_sb[:], d_bproj[:])
            bfc_sb = cp.tile([P, 32], FT, name="bfc_sb")
            nc.sync.dma_start(bfc_sb[:], d_bfc[:])
            bmp_sb = cp.tile([P, CT], FT, name="bmp_sb")
            nc.sync.dma_start(bmp_sb[:], d_bmp[:])
            b1d_sb = cp.tile([M, 1], FT, name="b1d_sb")
            nc.sync.dma_start(b1d_sb[:], d_b1d[:])
            b1u_sb = cp.tile([P, CT], FT, name="b1u_sb")
            nc.sync.dma_start(b1u_sb[:], d_b1u[:])
            b2d_sb = cp.tile([M, 1], FT, name="b2d_sb")
            nc.sync.dma_start(b2d_sb[:], d_b2d[:])
            b2u_sb = cp.tile([P, CT], FT, name="b2u_sb")
            nc.sync.dma_start(b2u_sb[:], d_b2u[:])
            ln1g_sb = cp.tile([P, CT], FT, name="ln1g_sb")
            nc.sync.dma_start(ln1g_sb[:], d_ln1g[:])
            ln1b_sb = cp.tile([P, CT], FT, name="ln1b_sb")
            nc.sync.dma_start(ln1b_sb[:], d_ln1b[:])
            ln2g_sb = cp.tile([P, CT], FT, name="ln2g_sb")
            nc.sync.dma_start(ln2g_sb[:], d_ln2g[:])
            ln2b_sb = cp.tile([P, CT], FT, name="ln2b_sb")
            nc.sync.dma_start(ln2b_sb[:], d_ln2b[:])
            # causal mask resident in SBUF (was re-streamed per (hp, kb):
            # 64 MB of HBM traffic that throttled the attention phase).
            # Allocated here; DMA'd after the K-half-1 loop so startup DMA
            # bandwidth goes to x and the first weight tiles.
            msk_sb = cp.tile([P, 16, R], BF, name="msk_sb")
            # Q weight tiles preloaded so the Q matmuls run with no DMA
            # dependency during the K/V AllGather window
            wq_all = cp.tile([P, CT, CT, P], BF, name="wq_all")

            # ---- residual stream (transposed), loaded once ----
            X = []
            for ct in range(CT):
                xt = rp.tile([P, R], FT, name=f"x{ct}")
                nc.sync.dma_start(xt[:], d_xT[:, ct, :])
                X.append(xt)

            STAGES = ['x', 'ln1', 'qkv', 'attn', 'proj', 'a1', 'mlp', 'full']
            s_idx = STAGES.index(stage)

            def want(s):
                return s_idx >= STAGES.index(s)

            def dump(tiles):
                for i, t in enumerate(tiles[:CT]):
                    if t[:].dtype == FT:
                        nc.sync.dma_start(d_out[i], t[:])
                    else:
                        nc.gpsimd.dma_start(d_out[i], t[:])

            def layer_norm(dst_pool, g_sb, b_sb, tag):
                """LN over the feature axis of the transposed stream."""
                s_ps = pp.tile([1, R], FT, name=f"s_{tag}", tag="mm")
                q_ps = pp.tile([1, R], FT, name=f"q_{tag}", tag="mm")
                xb = []
                for ct in range(CT):
                    xbt = sqp.tile([P, R], BF, name=f"xb_{tag}", tag="xb",
                                   bufs=CT)
                    nc.vector.tensor_copy(xbt[:], X[ct][:])
                    xb.append(xbt)
                    xsq = sqp.tile([P, R], BF, name=f"xsq_{tag}", tag="xsq")
                    nc.scalar.activation(xsq[:], X[ct][:], AF.Square)
                    nc.tensor.matmul(s_ps[:], _fr(ones[:]), _fr(xbt[:]),
                                     start=(ct == 0), stop=(ct == CT - 1))
                    nc.tensor.matmul(q_ps[:], _fr(ones[:]), _fr(xsq[:]),
                                     start=(ct == 0), stop=(ct == CT - 1))
                mu = lp.tile([1, R], BF, name=f"mu_{tag}", tag="ln")
                nc.scalar.mul(mu[:], s_ps[:], 1.0 / C)
                ex2 = lp.tile([1, R], FT, name=f"ex2_{tag}", tag="ln")
                nc.scalar.mul(ex2[:], q_ps[:], 1.0 / C)
                var = lp.tile([1, R], FT, name=f"var_{tag}", tag="ln")
                nc.vector.tensor_mul(var[:], mu[:], mu[:])
                nc.vector.tensor_sub(var[:], ex2[:], var[:])
                std = lp.tile([1, R], FT, name=f"std_{tag}", tag="ln")
                nc.scalar.activation(std[:], var[:], AF.Sqrt, bias=eps_sb[:])
                rstd = lp.tile([1, R], BF, name=f"rstd_{tag}", tag="ln")
                nc.vector.reciprocal(rstd[:], std[:])
                # broadcast mu and rstd across all 128 partitions (into PSUM)
                mu_b = pp.tile([P, R], FT, name=f"mub_{tag}", tag="mm")
                nc.tensor.matmul(mu_b[:], _fr(onesr[:]), _fr(mu[:]))
                rstd_b = pp.tile([P, R], FT, name=f"rsb_{tag}", tag="mm")
                nc.tensor.matmul(rstd_b[:], _fr(onesr[:]), _fr(rstd[:]))
                out_tiles = []
                for ct in range(CT):
                    xn = dst_pool.tile([P, R], BF, name=f"{tag}_{ct}", tag="xln")
                    nc.vector.tensor_sub(xn[:], X[ct][:], mu_b[:])
                    nc.vector.tensor_mul(xn[:], xn[:], rstd_b[:])
                    nc.scalar.activation(xn[:], xn[:], AF.Identity,
                                         scale=g_sb[:, ct:ct + 1],
                                         bias=b_sb[:, ct:ct + 1])
                    out_tiles.append(xn)
                return out_tiles

            with tc.tile_pool(name="qT", bufs=1) as qp, \
                 tc.tile_pool(name="oT", bufs=1) as op, \
                 tc.tile_pool(name="dram", bufs=1, space="DRAM") as dp:

                # K/V exchanged in four half-sized AllGathers (K heads 0-7,
                # V heads 0-7, K heads 8-15, V heads 8-15), each triggered as
                # soon as its half is computed so the slow ncfw collectives
                # pipeline under the remaining QKV matmuls and the first
                # attention head-pairs.
                # two half-sized K+V AllGathers: chunk h carries K and V for
                # heads [8h, 8h+8), triggered as soon as that half is
                # computed so the collectives pipeline under the remaining
                # QKV matmuls and the first attention head-pairs
                HC = RC // 2
                cc_i = [dp.tile([2 * HC], BF, name=f"cc_{i}i") for i in range(2)]
                cc_o = [dp.tile([4, 2 * HC], BF, name=f"cc_{i}o")
                        for i in range(2)]
                ccin_k = [t[0:HC].rearrange("(f t) -> f t", f=C // 2)
                          for t in cc_i]
                ccin_v = [t[HC:2 * HC].rearrange("(t f) -> t f", t=R)
                          for t in cc_i]
                # dummy collective fired immediately at core start: the
                # implicit pre-collective barrier then absorbs the multi-core
                # launch skew while startup DMAs run, instead of stalling the
                # first real gather
                cc_di = dp.tile([256], BF, name="cc_di")
                cc_do = dp.tile([8, 256], BF, name="cc_do")
                if want('qkv'):
                    nc.gpsimd.collective_compute(
                        "AllGather", OP.bypass,
                        replica_groups=[[0, 1, 2, 3, 4, 5, 6, 7]],
                        ins=[cc_di[:]], outs=[cc_do[:]])

                # ================= LN1 + QKV + AllGather =================
                with tc.tile_pool(name="xln1", bufs=CT) as x1p, \
                     tc.tile_pool(name="wq", bufs=3) as wqp, \
                     tc.tile_pool(name="wvp", bufs=2) as wvp, \
                     tc.tile_pool(name="kvb", bufs=4) as kvbp:

                    xln = (layer_norm(x1p, ln1g_sb, ln1b_sb, "ln1")
                           if want('ln1') else [])
                    if stage == 'ln1':
                        dump(xln)

                    def k_half(h):
                        # K features [512h, 512h+512) = heads [8h, 8h+8)
                        for ft in range(4 * h, 4 * h + 4):
                            mt = 8 + ft
                            wt = wqp.tile([P, CT, P], BF, name="wqk_t",
                                          tag="wqk")
                            nc.sync.dma_start(wt[:], d_wqk[mt])
                            ps = pp.tile([P, R], FT, name="ps_k", tag="mm")
                            for kt in range(CT):
                                nc.tensor.matmul(ps[:], _fr(wt[:, kt, :]),
                                                 _fr(xln[kt][:]),
                                                 start=(kt == 0),
                                                 stop=(kt == CT - 1))
                            kt_sb = kvbp.tile([P, R], BF, name="kt_sb",
                                              tag="kvb")
                            nc.scalar.activation(kt_sb[:], ps[:], AF.Identity,
                                                 bias=bqk_sb[:, mt:mt + 1])
                            lf = ft - 4 * h
                            nc.sync.dma_start(
                                ccin_k[h][P * lf:P * (lf + 1), :], kt_sb[:])

                    def v_half(nt):
                        wv_sb = wvp.tile([P, CT, 512], BF, name="wv_sb",
                                         tag="wv")
                        nc.sync.dma_start(wv_sb[:], d_wv[nt])
                        sel_v = sel_a if nt == 0 else sel_b
                        for mt in range(4):
                            ps = pp.tile([P, 512], FT, name="ps_v", tag="mm")
                            for kt in range(CT):
                                nc.tensor.matmul(
                                    ps[:],
                                    _fr(xln[kt][:, P * mt:P * (mt + 1)]),
                                    _fr(wv_sb[:, kt, :]),
                                    start=(kt == 0), stop=False)
                            # fold the (free-axis) V bias in as a rank-2
                            # broadcast matmul accumulated into the same bank
                            nc.tensor.matmul(ps[:], _fr(sel_v[:]),
                                             _fr(bv_sb[:]),
                                             start=False, stop=True)
                            v_sb = kvbp.tile([P, 512], BF, name="v_sb",
                                             tag="kvb")
                            nc.scalar.copy(v_sb[:], ps[:])
                            nc.sync.dma_start(
                                ccin_v[nt][P * mt:P * (mt + 1), :], v_sb[:])

                    def gather(ins, outs):
                        nc.gpsimd.collective_compute(
                            "AllGather", OP.bypass,
                            replica_groups=[[0, 1, 2, 3], [4, 5, 6, 7]],
                            ins=[ins[:]], outs=[outs[:]])

                    if want('qkv'):
                        k_half(0)
                        # deferred preloads: mask + Q weights land while the
                        # first half's V matmuls run
                        nc.sync.dma_start(msk_sb[:], d_mask[:])
                        nc.sync.dma_start(wq_all[:],
                                          d_wqk[0:8].transpose([1, 0, 2, 3]))
                        v_half(0)
                        gather(cc_i[0], cc_o[0])
                        k_half(1)
                        v_half(1)
                        gather(cc_i[1], cc_o[1])

                    # Q tiles (pre-scaled by 1/8 host-side), stay resident.
                    # Weights were preloaded, so these matmuls run during the
                    # AllGather with zero DMA dependencies.
                    qT = []
                    for mt in (range(CT) if want('qkv') else []):
                        ps = pp.tile([P, R], FT, name="ps_q", tag="mm")
                        for kt in range(CT):
                            nc.tensor.matmul(ps[:], _fr(wq_all[:, mt, kt, :]),
                                             _fr(xln[kt][:]),
                                             start=(kt == 0), stop=(kt == CT - 1))
                        qt = qp.tile([P, R], BF, name=f"q{mt}")
                        nc.scalar.activation(qt[:], ps[:], AF.Identity,
                                             bias=bqk_sb[:, mt:mt + 1])
                        qT.append(qt)
                    if stage == 'qkv':
                        dump(qT)

                # ======================= attention =======================
                with tc.tile_pool(name="kv", bufs=3) as kvp, \
                     tc.tile_pool(name="pexp", bufs=8) as pep, \
                     tc.tile_pool(name="maskp", bufs=6) as mp, \
                     tc.tile_pool(name="dsb", bufs=1) as dsp, \
                     tc.tile_pool(name="wp", bufs=1) as wpp:

                    # prefetch proj weights on the idle GpSimd DMA queue so
                    # they land during attention (the Sync queue stays free
                    # for the per-head K/V loads)
                    wp_all = wpp.tile([P, CT, CT, P], BF, name="wp_all")
                    if want('proj'):
                        nc.gpsimd.dma_start(
                            wp_all[:], d_wproj[:].transpose([1, 0, 2, 3]))
                    oT = []
                    den_all = dsp.tile([16, R], FT, name="den_all")
                    for hp in (range(CT) if want('attn') else []):
                        kT_h = kvp.tile([P, 16, P], BF, name="kT_h", tag="kT")
                        v_h = kvp.tile([P, 16, 130], BF, name="v_h", tag="vT")
                        # ones columns (64 and 129) for in-matmul denominators
                        nc.sync.dma_start(v_h[:, :, 64:65],
                                          vones_sb[:].unsqueeze(2))
                        nc.sync.dma_start(v_h[:, :, 129:130],
                                          vones_sb[:].unsqueeze(2))
                        half, lp_ = hp // 4, hp % 4
                        for r in range(4):
                            src_k = cc_o[half][r, 0:HC].rearrange(
                                "(f t) -> f t", f=C // 2)[
                                P * lp_:P * (lp_ + 1), :]
                            nc.sync.dma_start(
                                kT_h[:, 4 * r:4 * (r + 1), :],
                                src_k.rearrange("p (tb t) -> p tb t", tb=4))
                            src_v = cc_o[half][r, HC:2 * HC].rearrange(
                                "(tb p f) -> p tb f", tb=4, p=P)[
                                :, :, P * lp_:P * (lp_ + 1)]
                            nc.sync.dma_start(
                                v_h[:, 4 * r:4 * (r + 1), 0:64],
                                src_v[:, :, 0:64])
                            nc.sync.dma_start(
                                v_h[:, 4 * r:4 * (r + 1), 65:129],
                                src_v[:, :, 64:P])

                        o_ps0 = pp.tile([65, R], FT, name="o_ps0", tag="opsum",
                                        bufs=2)
                        o_ps1 = pp.tile([65, R], FT, name="o_ps1", tag="opsum",
                                        bufs=2)
                        for kb in range(16):
                            s0 = pp.tile([P, 2 * R], FT, name="s0", tag="spsum",
                                         bufs=2)
                            nc.tensor.matmul(s0[:, 0:R], _fr(kT_h[0:64, kb, :]),
                                             _fr(qT[hp][0:64, :]))
                            nc.tensor.matmul(s0[:, R:2 * R],
                                             _fr(kT_h[64:P, kb, :]),
                                             _fr(qT[hp][64:P, :]))
                            p0 = pep.tile([P, 2 * R], BF, name="p0", tag="pexp")
                            nc.scalar.activation(p0[:], s0[:], AF.Exp)
                            nc.vector.tensor_mul(p0[:, 0:R], p0[:, 0:R],
                                                 msk_sb[:, kb, :])
                            nc.vector.tensor_mul(p0[:, R:2 * R], p0[:, R:2 * R],
                                                 msk_sb[:, kb, :])
                            nc.tensor.matmul(o_ps0[:],
                                             _fr(v_h[:, kb, 0:65]),
                                             _fr(p0[:, 0:R]),
                                             start=(kb == 0), stop=(kb == 15))
                            nc.tensor.matmul(o_ps1[:],
                                             _fr(v_h[:, kb, 65:130]),
                                             _fr(p0[:, R:2 * R]),
                                             start=(kb == 0), stop=(kb == 15))

                        # denominators ride along as psum row 64
                        dt0 = mp.tile([1, R], FT, name="dt0", tag="dt", bufs=4)
                        dt1 = mp.tile([1, R], FT, name="dt1", tag="dt", bufs=4)
                        nc.scalar.copy(dt0[:], o_ps0[64:65, :])
                        nc.scalar.copy(dt1[:], o_ps1[64:65, :])
                        nc.sync.dma_start(den_all[2 * hp:2 * hp + 1, :], dt0[:])
                        nc.sync.dma_start(
                            den_all[2 * hp + 1:2 * hp + 2, :], dt1[:])
                        ot = op.tile([P, R], BF, name=f"o{hp}")
                        nc.vector.tensor_copy(ot[0:64, :], o_ps0[0:64, :])
                        nc.vector.tensor_copy(ot[64:P, :], o_ps1[0:64, :])
                        oT.append(ot)

                    if want('attn'):
                        # one batched reciprocal for all 16 head denominators
                        rden = dsp.tile([16, R], BF, name="rden")
                        nc.vector.reciprocal(rden[:], den_all[:])
                        for hp in range(CT):
                            rdb = pp.tile([P, R], FT, name="rdb", tag="mm")
                            nc.tensor.matmul(rdb[:], sel16_sb[:, hp, :],
                                             rden[:])
                            nc.vector.tensor_mul(oT[hp][:], oT[hp][:], rdb[:])
                    if stage == 'attn':
                        dump(oT)

                    # ================ proj + residual ================
                    for mt in (range(CT) if want('proj') else []):
                        ps = pp.tile([P, R], FT, name="ps_pr", tag="mm")
                        for kt in range(CT):
                            nc.tensor.matmul(ps[:], _fr(wp_all[:, mt, kt, :]),
                                             _fr(oT[kt][:]),
                                             start=(kt == 0), stop=(kt == CT - 1))
                        nc.vector.scalar_tensor_tensor(
                            X[mt][:], ps[:], bproj_sb[:, mt:mt + 1], X[mt][:],
                            op0=OP.add, op1=OP.add)

            # ================== adapters + MLP ==================
            def adapter(d_dw, d_uw, bd_sb, bu_sb, tag):
                with tc.tile_pool(name=f"ad_{tag}", bufs=1) as adp:
                    ad = adp.tile([P, CT, M], BF, name=f"ad_{tag}")
                    nc.sync.dma_start(ad[:], d_dw[:])
                    au = adp.tile([M, CT, P], BF, name=f"au_{tag}")
                    nc.sync.dma_start(au[:], d_uw[:])
                    ps_a = pp.tile([M, R], FT, name=f"psa_{tag}", tag="mm")
                    for kt in range(CT):
                        xbt = adp.tile([P, R], BF, name=f"xb_{tag}", tag="xb",
                                       bufs=3)
                        nc.vector.tensor_copy(xbt[:], X[kt][:])
                        nc.tensor.matmul(ps_a[:], _fr(ad[:, kt, :]),
                                         _fr(xbt[:]),
                                         start=(kt == 0), stop=(kt == CT - 1))
                    ar = adp.tile([M, R], BF, name=f"ar_{tag}")
                    nc.scalar.activation(ar[:], ps_a[:], AF.Relu,
                                         bias=bd_sb[:, 0:1])
                    for mt in range(CT):
                        ps = pp.tile([P, R], FT, name=f"psu_{tag}", tag="mm")
                        nc.tensor.matmul(ps[:], _fr(au[:, mt, :]), _fr(ar[:]))
                        nc.vector.scalar_tensor_tensor(
                            X[mt][:], ps[:], bu_sb[:, mt:mt + 1], X[mt][:],
                            op0=OP.add, op1=OP.add)

            if want('a1'):
                adapter(d_a1d, d_a1u, b1d_sb, b1u_sb, "a1")

            # ---------------- LN2 + MLP ----------------
            with tc.tile_pool(name="xln2", bufs=CT) as x2p, \
                 tc.tile_pool(name="wfcp", bufs=3) as wfp, \
                 tc.tile_pool(name="hT", bufs=32) as hp_, \
                 tc.tile_pool(name="wmpp", bufs=2) as wmp:

                x2 = layer_norm(x2p, ln2g_sb, ln2b_sb, "ln2") if want('mlp') else []
                hT = []
                for mt in (range(32) if want('mlp') else []):
                    wt = wfp.tile([P, CT, P], BF, name="wfc_t", tag="wfc")
                    nc.sync.dma_start(wt[:], d_wfc[mt])
                    ps = pp.tile([P, R], FT, name="ps_fc", tag="mm")
                    for kt in range(CT):
                        nc.tensor.matmul(ps[:], _fr(wt[:, kt, :]), _fr(x2[kt][:]),
                                         start=(kt == 0), stop=(kt == CT - 1))
                    ht = hp_.tile([P, R], BF, name="ht", tag="hT")
                    nc.scalar.activation(ht[:], ps[:], AF.Gelu_apprx_tanh,
                                         bias=bfc_sb[:, mt:mt + 1])
                    hT.append(ht)
                for mt in (range(CT) if want('mlp') else []):
                    wt = wmp.tile([P, 32, P], BF, name="wmp_t", tag="wmp")
                    nc.sync.dma_start(wt[:], d_wmp[mt])
                    ps = pp.tile([P, R], FT, name="ps_mp", tag="mm")
                    for kt in range(32):
                        nc.tensor.matmul(ps[:], _fr(wt[:, kt, :]), _fr(hT[kt][:]),
                                         start=(kt == 0), stop=(kt == 31))
                    nc.vector.scalar_tensor_tensor(
                        X[mt][:], ps[:], bmp_sb[:, mt:mt + 1], X[mt][:],
                        op0=OP.add, op1=OP.add)

            if want('full'):
                adapter(d_a2d, d_a2u, b2d_sb, b2u_sb, "a2")

            # ---------------- output ----------------
            if stage in ('x', 'proj', 'a1', 'mlp', 'full'):
                for ct in range(CT):
                    nc.sync.dma_start(d_out[ct], X[ct][:])

    nc.compile()
    return nc


def _prep_shared(inputs):
    """Host-side tiling of weights/biases into the DRAM layouts above."""
    f32 = np.float32
    bf16 = ml_dtypes.bfloat16
    W = {k: np.ascontiguousarray(np.asarray(v, dtype=f32))
         for k, v in inputs.items()}
    s = f32(1.0 / np.sqrt(HD))
    aw = W['attn_w']
    ab = W['attn_b']
    wq = aw[:, :C] * s
    wk = aw[:, C:2 * C]
    wv = aw[:, 2 * C:]

    def lhst_tiles(w, nmt):
        # w [K, Mout] -> [nmt, P, K//P, P]: tile[mt, p, kt, m] = w[P*kt+p, P*mt+m]
        kk, mm = w.shape
        return np.ascontiguousarray(
            w.reshape(kk // P, P, nmt, P).transpose(2, 1, 0, 3))

    wqk = np.concatenate([lhst_tiles(wq, 8), lhst_tiles(wk, 8)], axis=0)
    # wv moving tiles [2, P, CT, 512]: tile[nt, p, kt, m] = wv[P*kt+p, 512*nt+m]
    wv_m = np.ascontiguousarray(
        wv.reshape(CT, P, 2, 512).transpose(2, 1, 0, 3))

    def col_vec(v, nmt):
        # v [nmt*P] -> [P, nmt]
        return np.ascontiguousarray(v.reshape(nmt, P).T)

    shared = {
        'wqk': wqk,
        'wv': wv_m,
        'wproj': lhst_tiles(W['proj_w'], CT),
        'wfc': lhst_tiles(W['fc_w'], 32),
        'wmp': lhst_tiles(W['mlp_pw'], CT),
        'a1d': np.ascontiguousarray(
            W['a1_dw'].reshape(CT, P, M).transpose(1, 0, 2)),
        'a1u': np.ascontiguousarray(W['a1_uw'].reshape(M, CT, P)),
        'a2d': np.ascontiguousarray(
            W['a2_dw'].reshape(CT, P, M).transpose(1, 0, 2)),
        'a2u': np.ascontiguousarray(W['a2_uw'].reshape(M, CT, P)),
        'bqk': np.ascontiguousarray(
            np.concatenate([ab[:C] * s, ab[C:2 * C]]).reshape(16, P).T),
        'bv': np.ascontiguousarray(ab[2 * C:].reshape(2, 512)),
        'bproj': col_vec(W['proj_b'], CT),
        'bfc': col_vec(W['fc_b'], 32),
        'bmp': col_vec(W['mlp_pb'], CT),
        'b1d': np.ascontiguousarray(W['a1_db'].reshape(M, 1)),
        'b1u': col_vec(W['a1_ub'], CT),
        'b2d': np.ascontiguousarray(W['a2_db'].reshape(M, 1)),
        'b2u': col_vec(W['a2_ub'], CT),
        'ln1g': col_vec(W['ln1_g'], CT),
        'ln1b': col_vec(W['ln1_b'], CT),
        'ln2g': col_vec(W['ln2_g'], CT),
        'ln2b': col_vec(W['ln2_b'], CT),
    }
    selc = np.zeros((2, 5 * P), dtype=f32)
    selc[0, 0:P] = 1.0                       # sel_a: row 0 everywhere
    selc[1, P:2 * P] = 1.0                   # sel_b: row 1 everywhere
    selc[0, 2 * P:2 * P + 64] = 1.0          # sel_h: row m//64
    selc[1, 2 * P + 64:3 * P] = 1.0
    selc[0, 3 * P:3 * P + 64] = 1.0          # sel_lo: partitions 0:64
    selc[0, 4 * P + 64:5 * P] = 1.0          # sel_hi: partitions 64:128
    shared['selc'] = selc
    shared['onesc'] = np.ones((P, 1), dtype=f32)
    shared['onesr'] = np.ones((1, P), dtype=f32)
    shared['vones'] = np.ones((P, 16), dtype=f32)
    sel16 = np.zeros((CT, 16, P), dtype=f32)
    for hp in range(CT):
        sel16[hp, 2 * hp, 0:64] = 1.0
        sel16[hp, 2 * hp + 1, 64:P] = 1.0
    shared['sel16'] = sel16
    for k in ('wqk', 'wv', 'wproj', 'wfc', 'wmp', 'a1d', 'a1u', 'a2d', 'a2u',
              'bv', 'selc', 'onesc', 'onesr', 'vones', 'sel16'):
        shared[k] = np.ascontiguousarray(shared[k].astype(bf16))
    return shared


def _prep_core(x, c):
    b, lc = c // 4, c % 4
    xl = np.asarray(x[b, R * lc:R * (lc + 1), :], dtype=np.float32)   # [R, C]
    xT = np.ascontiguousarray(
        xl.T.reshape(CT, P, R).transpose(1, 0, 2))                    # [P, CT, R]
    kj = np.arange(T)[:, None]
    qi = np.arange(R)[None, :] + R * lc
    mask = (kj <= qi).astype(np.float32)                              # [T, R]
    m3 = mask.reshape(16, P, R).transpose(1, 0, 2)                    # [P, 16, R]
    maskT = np.ascontiguousarray(m3.astype(ml_dtypes.bfloat16))
    return {'xT': xT, 'mask': maskT}


def _run(inputs, trace=False, stage='full'):
    if stage not in _CACHE:
        _CACHE[stage] = _build(stage)
    nc = _CACHE[stage]
    shared = _prep_shared(inputs)
    x = np.asarray(inputs['x'], dtype=np.float32)
    in_maps = []
    for c in range(NCORES):
        m = dict(shared)
        m.update(_prep_core(x, c))
        in_maps.append(m)
    kwargs = {}
    if trace:
        from trn_agent_boot.trn_boot import _ntff_profile_via_ctypes
        hook = _ntff_profile_via_ctypes('/opt/axon/libaxon_pjrt.so')
        mod = types.ModuleType('antenv.axon_hooks')
        mod.get_axon_ntff_profile_hook = lambda: hook
        sys.modules['antenv.axon_hooks'] = mod
        bass_utils.upload_artifacts = lambda tmpdir: "/tmp/no-upload"
        kwargs['trace'] = True
    res = bass_utils.run_bass_kernel_spmd(
        nc, in_maps, core_ids=list(range(NCORES)), **kwargs)
    y = np.zeros((B, T, C), dtype=np.float32)
    for c in range(NCORES):
        b, lc = c // 4, c % 4
        o = res.results[c]['out']          # [CT, P, R]
        y[b, R * lc:R * (lc + 1), :] = o.reshape(C, R).T
    return y, res


def kernel(**inputs):
    y, _ = _run(inputs, trace=False)
    return y



# revision 33
# speedup vs baseline: 1.1328x; 1.0438x over previous
"""Trainium2 Bass kernel for nn_BlockWithAdapter (B=2, T=2048, C=1024, H=16, M=64).

Strategy: token-parallel over 8 cores (core c owns 512 tokens of batch c//4).
Residual stream kept transposed [feature, token] on-chip so every linear layer
is a natural TensorE matmul (weights stationary, activations moving, N=512).
Causal attention computed dense per core (512 q x 2048 k, all 16 heads) with a
per-core multiplicative {0,1} mask applied after exp; softmax without
max-subtraction (scores are unit-scale by construction). K.T and V(plain) are
AllGathered within each batch group of 4 cores — the only collective.
All matmuls run as float32r (full fp32 data, full-rate PE mode at N>=256).
"""
import sys
import types

sys.path.insert(0, '/opt/trn_rl_repo')

import ml_dtypes
import numpy as np

import concourse.bass as bass  # noqa: F401  (registers arch)
import concourse.mybir as mybir
import concourse.tile as tile
from concourse import bacc
from concourse import bass_utils

P = 128
B, T, C, H, M = 2, 2048, 1024, 16, 64
HD = C // H            # 64
R = 512                # tokens per core
CT = C // P            # 8 feature tiles of the residual stream
NCORES = 8
RC = R * C             # floats in one K.T (or V) local buffer
EPS = 1e-5

FT = mybir.dt.float32
FR = mybir.dt.float32r
BF = mybir.dt.bfloat16
AF = mybir.ActivationFunctionType
OP = mybir.AluOpType

_CACHE = {}


def _fr(ap):
    return ap


def _build(stage='full'):
    nc = bacc.Bacc("TRN2", target_bir_lowering=False, debug=False,
                   num_devices=NCORES)

    # ---- DRAM I/O (per-core inputs; weights replicated across cores) ----
    d_xT = nc.dram_tensor("xT", [P, CT, R], FT, kind="ExternalInput")
    d_mask = nc.dram_tensor("mask", [P, 16, R], BF, kind="ExternalInput")
    d_wqk = nc.dram_tensor("wqk", [16, P, CT, P], BF, kind="ExternalInput")
    d_wv = nc.dram_tensor("wv", [2, P, CT, 512], BF, kind="ExternalInput")
    d_wproj = nc.dram_tensor("wproj", [CT, P, CT, P], BF, kind="ExternalInput")
    d_wfc = nc.dram_tensor("wfc", [32, P, CT, P], BF, kind="ExternalInput")
    d_wmp = nc.dram_tensor("wmp", [CT, P, 32, P], BF, kind="ExternalInput")
    d_a1d = nc.dram_tensor("a1d", [P, CT, M], BF, kind="ExternalInput")
    d_a1u = nc.dram_tensor("a1u", [M, CT, P], BF, kind="ExternalInput")
    d_a2d = nc.dram_tensor("a2d", [P, CT, M], BF, kind="ExternalInput")
    d_a2u = nc.dram_tensor("a2u", [M, CT, P], BF, kind="ExternalInput")
    d_bqk = nc.dram_tensor("bqk", [P, 16], FT, kind="ExternalInput")
    d_bv = nc.dram_tensor("bv", [2, 512], BF, kind="ExternalInput")
    d_bproj = nc.dram_tensor("bproj", [P, CT], FT, kind="ExternalInput")
    d_bfc = nc.dram_tensor("bfc", [P, 32], FT, kind="ExternalInput")
    d_bmp = nc.dram_tensor("bmp", [P, CT], FT, kind="ExternalInput")
    d_b1d = nc.dram_tensor("b1d", [M, 1], FT, kind="ExternalInput")
    d_b1u = nc.dram_tensor("b1u", [P, CT], FT, kind="ExternalInput")
    d_b2d = nc.dram_tensor("b2d", [M, 1], FT, kind="ExternalInput")
    d_b2u = nc.dram_tensor("b2u", [P, CT], FT, kind="ExternalInput")
    d_ln1g = nc.dram_tensor("ln1g", [P, CT], FT, kind="ExternalInput")
    d_ln1b = nc.dram_tensor("ln1b", [P, CT], FT, kind="ExternalInput")
    d_ln2g = nc.dram_tensor("ln2g", [P, CT], FT, kind="ExternalInput")
    d_ln2b = nc.dram_tensor("ln2b", [P, CT], FT, kind="ExternalInput")
    d_selc = nc.dram_tensor("selc", [2, 5 * P], BF, kind="ExternalInput")
    d_ones = nc.dram_tensor("onesc", [P, 1], BF, kind="ExternalInput")
    d_onesr = nc.dram_tensor("onesr", [1, P], BF, kind="ExternalInput")
    d_vones = nc.dram_tensor("vones", [P, 16], BF, kind="ExternalInput")
    d_sel16 = nc.dram_tensor("sel16", [CT, 16, P], BF, kind="ExternalInput")
    d_out = nc.dram_tensor("out", [CT, P, R], FT, kind="ExternalOutput")

    with tile.TileContext(nc) as tc, \
         nc.allow_low_precision(reason="float32r is 4-byte fp32-rounded"):
        with tc.tile_pool(name="const", bufs=1) as cp, \
             tc.tile_pool(name="resid", bufs=1) as rp, \
             tc.tile_pool(name="psum", bufs=2, space="PSUM") as pp, \
             tc.tile_pool(name="lnsb", bufs=4) as lp, \
             tc.tile_pool(name="sq", bufs=3) as sqp:

            # ---- constants ----
            ones = cp.tile([P, 1], BF, name="ones")
            nc.sync.dma_start(ones[:], d_ones[:])
            onesr = cp.tile([1, P], BF, name="onesr")
            nc.sync.dma_start(onesr[:], d_onesr[:])
            vones_sb = cp.tile([P, 16], BF, name="vones_sb")
            nc.sync.dma_start(vones_sb[:], d_vones[:])
            sel16_sb = cp.tile([16, CT, P], BF, name="sel16_sb")
            nc.sync.dma_start(sel16_sb[:], d_sel16[:].transpose([1, 0, 2]))
            onesf = cp.tile([P, 1], FT, name="onesf")
            nc.vector.memset(onesf[:], 1.0)
            eps_sb = cp.tile([1, 1], FT, name="eps_sb")
            nc.vector.memset(eps_sb[:], EPS)
            # selector matrices for broadcasting a row vector across
            # partitions via a K=2 matmul: out[m, :] = rhs[row(m), :]
            # (host-staged; memset cannot write partition base 1)
            selc_sb = cp.tile([2, 5 * P], BF, name="selc_sb")
            nc.sync.dma_start(selc_sb[:], d_selc[:])
            sel_a = selc_sb[:, 0:P]          # row 0 -> all 128 partitions
            sel_b = selc_sb[:, P:2 * P]      # row 1 -> all 128 partitions
            sel_h = selc_sb[:, 2 * P:3 * P]  # row m//64
            sel_lo = selc_sb[0:1, 3 * P:4 * P]   # row 0 -> partitions 0:64
            sel_hi = selc_sb[0:1, 4 * P:5 * P]   # row 0 -> partitions 64:128
            bqk_sb = cp.tile([P, 16], FT, name="bqk_sb")
            nc.sync.dma_start(bqk_sb[:], d_bqk[:])
            bv_sb = cp.tile([2, 512], BF, name="bv_sb")
            nc.sync.dma_start(bv_sb[:], d_bv[:])
            bproj_sb = cp.tile([P, CT], FT, name="bproj_sb")
            nc.sync.dma_start(bproj_sb[:], d_bproj[:])
            bfc_sb = cp.tile([P, 32], FT, name="bfc_sb")
            nc.sync.dma_start(bfc_sb[:], d_bfc[:])
            bmp_sb = cp.tile([P, CT], FT, name="bmp_sb")
            nc.sync.dma_start(bmp_sb[:], d_bmp[:])
            b1d_sb = cp.tile([M, 1], FT, name="b1d_sb")
            nc.sync.dma_start(b1d_sb[:], d_b1d[:])
            b1u_sb = cp.tile([P, CT], FT, name="b1u_sb")
            nc.sync.dma_start(b1u_sb[:], d_b1u[:])
            b2d_sb = cp.tile([M, 1], FT, name="b2d_sb")
            nc.sync.dma_start(b2d_sb[:], d_b2d[:])
            b2u_sb = cp.tile([P, CT], FT, name="b2u_sb")
            nc.sync.dma_start(b2u_sb[:], d_b2u[:])
            ln1g_sb = cp.tile([P, CT], FT, name="ln1g_sb")
            nc.sync.dma_start(ln1g_sb[:], d_ln1g[:])
            ln1b_sb = cp.tile([P, CT], FT, name="ln1b_sb")
            nc.sync.dma_start(ln1b_sb[:], d_ln1b[:])
            ln2g_sb = cp.tile([P, CT], FT, name="ln2g_sb")
            nc.sync.dma_start(ln2g_sb[:], d_ln2g[:])
            ln2b_sb = cp.tile([P, CT], FT, name="ln2b_sb")
            nc.sync.dma_start(ln2b_sb[:], d_ln2b[:])
            # causal mask resident in SBUF (was re-streamed per (hp, kb):
            # 64 MB of HBM traffic that throttled the attention phase).
            # Allocated here; DMA'd after the K-half-1 loop so startup DMA
            # bandwidth goes to x and the first weight tiles.
            msk_sb = cp.tile([P, 16, R], BF, name="msk_sb")
            # Q weight tiles preloaded so the Q matmuls run with no DMA
            # dependency during the K/V AllGather window
            wq_all = cp.tile([P, CT, CT, P], BF, name="wq_all")

            # ---- residual stream (transposed), loaded once ----
            X = []
            for ct in range(CT):
                xt = rp.tile([P, R], FT, name=f"x{ct}")
                nc.sync.dma_start(xt[:], d_xT[:, ct, :])
                X.append(xt)

            STAGES = ['x', 'ln1', 'qkv', 'attn', 'proj', 'a1', 'mlp', 'full']
            s_idx = STAGES.index(stage)

            def want(s):
                return s_idx >= STAGES.index(s)

            def dump(tiles):
                for i, t in enumerate(tiles[:CT]):
                    if t[:].dtype == FT:
                        nc.sync.dma_start(d_out[i], t[:])
                    else:
                        nc.gpsimd.dma_start(d_out[i], t[:])

            def layer_norm(dst_pool, g_sb, b_sb, tag):
                """LN over the feature axis of the transposed stream."""
                s_ps = pp.tile([1, R], FT, name=f"s_{tag}", tag="mm")
                q_ps = pp.tile([1, R], FT, name=f"q_{tag}", tag="mm")
                xb = []
                for ct in range(CT):
                    xbt = sqp.tile([P, R], BF, name=f"xb_{tag}", tag="xb",
                                   bufs=CT)
                    nc.vector.tensor_copy(xbt[:], X[ct][:])
                    xb.append(xbt)
                    xsq = sqp.tile([P, R], BF, name=f"xsq_{tag}", tag="xsq")
                    nc.scalar.activation(xsq[:], X[ct][:], AF.Square)
                    nc.tensor.matmul(s_ps[:], _fr(ones[:]), _fr(xbt[:]),
                                     start=(ct == 0), stop=(ct == CT - 1))
                    nc.tensor.matmul(q_ps[:], _fr(ones[:]), _fr(xsq[:]),
                                     start=(ct == 0), stop=(ct == CT - 1))
                mu = lp.tile([1, R], BF, name=f"mu_{tag}", tag="ln")
                nc.scalar.mul(mu[:], s_ps[:], 1.0 / C)
                ex2 = lp.tile([1, R], FT, name=f"ex2_{tag}", tag="ln")
                nc.scalar.mul(ex2[:], q_ps[:], 1.0 / C)
                var = lp.tile([1, R], FT, name=f"var_{tag}", tag="ln")
                nc.vector.tensor_mul(var[:], mu[:], mu[:])
                nc.vector.tensor_sub(var[:], ex2[:], var[:])
                std = lp.tile([1, R], FT, name=f"std_{tag}", tag="ln")
                nc.scalar.activation(std[:], var[:], AF.Sqrt, bias=eps_sb[:])
                rstd = lp.tile([1, R], BF, name=f"rstd_{tag}", tag="ln")
                nc.vector.reciprocal(rstd[:], std[:])
                # broadcast mu and rstd across all 128 partitions (into PSUM)
                mu_b = pp.tile([P, R], FT, name=f"mub_{tag}", tag="mm")
                nc.tensor.matmul(mu_b[:], _fr(onesr[:]), _fr(mu[:]))
                rstd_b = pp.tile([P, R], FT, name=f"rsb_{tag}", tag="mm")
                nc.tensor.matmul(rstd_b[:], _fr(onesr[:]), _fr(rstd[:]))
                out_tiles = []
                for ct in range(CT):
                    xn = dst_pool.tile([P, R], BF, name=f"{tag}_{ct}", tag="xln")
                    nc.vector.tensor_sub(xn[:], X[ct][:], mu_b[:])
                    nc.vector.tensor_mul(xn[:], xn[:], rstd_b[:])
                    nc.scalar.activation(xn[:], xn[:], AF.Identity,
                                         scale=g_sb[:, ct:ct + 1],
                                         bias=b_sb[:, ct:ct + 1])
                    out_tiles.append(xn)
                return out_tiles

            with tc.tile_pool(name="qT", bufs=1) as qp, \
                 tc.tile_pool(name="oT", bufs=1) as op, \
                 tc.tile_pool(name="dram", bufs=1, space="DRAM") as dp:

                # K/V exchanged in four half-sized AllGathers (K heads 0-7,
                # V heads 0-7, K heads 8-15, V heads 8-15), each triggered as
                # soon as its half is computed so the slow ncfw collectives
                # pipeline under the remaining QKV matmuls and the first
                # attention head-pairs.
                # two half-sized K+V AllGathers: chunk h carries K and V for
                # heads [8h, 8h+8), triggered as soon as that half is
                # computed so the collectives pipeline under the remaining
                # QKV matmuls and the first attention head-pairs
                HC = RC // 2
                cc_i = [dp.tile([2 * HC], BF, name=f"cc_{i}i") for i in range(2)]
                cc_o = [dp.tile([4, 2 * HC], BF, name=f"cc_{i}o")
                        for i in range(2)]
                ccin_k = [t[0:HC].rearrange("(f t) -> f t", f=C // 2)
                          for t in cc_i]
                ccin_v = [t[HC:2 * HC].rearrange("(t f) -> t f", t=R)
                          for t in cc_i]
                # dummy collective fired immediately at core start: the
                # implicit pre-collective barrier then absorbs the multi-core
                # launch skew while startup DMAs run, instead of stalling the
                # first real gather
                cc_di = dp.tile([256], BF, name="cc_di")
                cc_do = dp.tile([8, 256], BF, name="cc_do")
                if want('qkv'):
                    nc.gpsimd.collective_compute(
                        "AllGather", OP.bypass,
                        replica_groups=[[0, 1, 2, 3, 4, 5, 6, 7]],
                        ins=[cc_di[:]], outs=[cc_do[:]])

                # ================= LN1 + QKV + AllGather =================
                with tc.tile_pool(name="xln1", bufs=CT) as x1p, \
                     tc.tile_pool(name="wq", bufs=3) as wqp, \
                     tc.tile_pool(name="wvp", bufs=2) as wvp, \
                     tc.tile_pool(name="kvb", bufs=4) as kvbp:

                    xln = (layer_norm(x1p, ln1g_sb, ln1b_sb, "ln1")
                           if want('ln1') else [])
                    if stage == 'ln1':
                        dump(xln)

                    def k_half(h):
                        # K features [512h, 512h+512) = heads [8h, 8h+8)
                        for ft in range(4 * h, 4 * h + 4):
                            mt = 8 + ft
                            wt = wqp.tile([P, CT, P], BF, name="wqk_t",
                                          tag="wqk")
                            nc.sync.dma_start(wt[:], d_wqk[mt])
                            ps = pp.tile([P, R], FT, name="ps_k", tag="mm")
                            for kt in range(CT):
                                nc.tensor.matmul(ps[:], _fr(wt[:, kt, :]),
                                                 _fr(xln[kt][:]),
                                                 start=(kt == 0),
                                                 stop=(kt == CT - 1))
                            kt_sb = kvbp.tile([P, R], BF, name="kt_sb",
                                              tag="kvb")
                            nc.scalar.activation(kt_sb[:], ps[:], AF.Identity,
                                                 bias=bqk_sb[:, mt:mt + 1])
                            lf = ft - 4 * h
                            nc.sync.dma_start(
                                ccin_k[h][P * lf:P * (lf + 1), :], kt_sb[:])

                    def v_half(nt):
                        wv_sb = wvp.tile([P, CT, 512], BF, name="wv_sb",
                                         tag="wv")
                        nc.sync.dma_start(wv_sb[:], d_wv[nt])
                        sel_v = sel_a if nt == 0 else sel_b
                        for mt in range(4):
                            ps = pp.tile([P, 512], FT, name="ps_v", tag="mm")
                            for kt in range(CT):
                                nc.tensor.matmul(
                                    ps[:],
                                    _fr(xln[kt][:, P * mt:P * (mt + 1)]),
                                    _fr(wv_sb[:, kt, :]),
                                    start=(kt == 0), stop=False)
                            # fold the (free-axis) V bias in as a rank-2
                            # broadcast matmul accumulated into the same bank
                            nc.tensor.matmul(ps[:], _fr(sel_v[:]),
                                             _fr(bv_sb[:]),
                                             start=False, stop=True)
                            v_sb = kvbp.tile([P, 512], BF, name="v_sb",
                                             tag="kvb")
                            nc.scalar.copy(v_sb[:], ps[:])
                            nc.sync.dma_start(
                                ccin_v[nt][P * mt:P * (mt + 1), :], v_sb[:])

                    def gather(ins, outs):
                        nc.gpsimd.collective_compute(
                            "AllGather", OP.bypass,
                            replica_groups=[[0, 1, 2, 3], [4, 5, 6, 7]],
                            ins=[ins[:]], outs=[outs[:]])

                    if want('qkv'):
                        k_half(0)
                        # deferred preloads: mask + Q weights land while the
                        # first half's V matmuls run
                        nc.sync.dma_start(msk_sb[:], d_mask[:])
                        nc.sync.dma_start(wq_all[:],
                                          d_wqk[0:8].transpose([1, 0, 2, 3]))
                        v_half(0)
                        gather(cc_i[0], cc_o[0])
                        k_half(1)
                        v_half(1)
                        gather(cc_i[1], cc_o[1])

                    # Q tiles (pre-scaled by 1/8 host-side), stay resident.
                    # Weights were preloaded, so these matmuls run during the
                    # AllGather with zero DMA dependencies.
                    qT = []
                    for mt in (range(CT) if want('qkv') else []):
                        ps = pp.tile([P, R], FT, name="ps_q", tag="mm")
                        for kt in range(CT):
                            nc.tensor.matmul(ps[:], _fr(wq_all[:, mt, kt, :]),
                                             _fr(xln[kt][:]),
                                             start=(kt == 0), stop=(kt == CT - 1))
                        qt = qp.tile([P, R], BF, name=f"q{mt}")
                        nc.scalar.activation(qt[:], ps[:], AF.Identity,
                                             bias=bqk_sb[:, mt:mt + 1])
                        qT.append(qt)
                    if stage == 'qkv':
                        dump(qT)

                # ======================= attention =======================
                with tc.tile_pool(name="kv", bufs=3) as kvp, \
                     tc.tile_pool(name="pexp", bufs=8) as pep, \
                     tc.tile_pool(name="maskp", bufs=6) as mp, \
                     tc.tile_pool(name="dsb", bufs=1) as dsp, \
                     tc.tile_pool(name="wp", bufs=1) as wpp:

                    # prefetch proj weights on the idle GpSimd DMA queue so
                    # they land during attention (the Sync queue stays free
                    # for the per-head K/V loads)
                    wp_all = wpp.tile([P, CT, CT, P], BF, name="wp_all")
                    if want('proj'):
                        nc.gpsimd.dma_start(
                            wp_all[:], d_wproj[:].transpose([1, 0, 2, 3]))
                    oT = []
                    den_all = dsp.tile([16, R], FT, name="den_all")
                    for hp in (range(CT) if want('attn') else []):
                        kT_h = kvp.tile([P, 16, P], BF, name="kT_h", tag="kT")
                        v_h = kvp.tile([P, 16, 130], BF, name="v_h", tag="vT")
                        # ones columns (64 and 129) for in-matmul denominators
                        nc.sync.dma_start(v_h[:, :, 64:65],
                                          vones_sb[:].unsqueeze(2))
                        nc.sync.dma_start(v_h[:, :, 129:130],
                                          vones_sb[:].unsqueeze(2))
                        half, lp_ = hp // 4, hp % 4
                        for r in range(4):
                            src_k = cc_o[half][r, 0:HC].rearrange(
                                "(f t) -> f t", f=C // 2)[
                                P * lp_:P * (lp_ + 1), :]
                            nc.sync.dma_start(
                                kT_h[:, 4 * r:4 * (r + 1), :],
                                src_k.rearrange("p (tb t) -> p tb t", tb=4))
                            src_v = cc_o[half][r, HC:2 * HC].rearrange(
                                "(tb p f) -> p tb f", tb=4, p=P)[
                                :, :, P * lp_:P * (lp_ + 1)]
                            nc.sync.dma_start(
                                v_h[:, 4 * r:4 * (r + 1), 0:64],
                                src_v[:, :, 0:64])
                            nc.sync.dma_start(
                                v_h[:, 4 * r:4 * (r + 1), 65:129],
                                src_v[:, :, 64:P])

                        o_ps0 = pp.tile([65, R], FT, name="o_ps0", tag="opsum",
                                        bufs=2)
                        o_ps1 = pp.tile([65, R], FT, name="o_ps1", tag="opsum",
                                        bufs=2)
                        for kb in range(16):
                            s0 = pp.tile([P, 2 * R], FT, name="s0", tag="spsum",
                                         bufs=2)
                            nc.tensor.matmul(s0[:, 0:R], _fr(kT_h[0:64, kb, :]),
                                             _fr(qT[hp][0:64, :]))
                            nc.tensor.matmul(s0[:, R:2 * R],
                                             _fr(kT_h[64:P, kb, :]),
                                             _fr(qT[hp][64:P, :]))
                            p0 = pep.tile([P, 2 * R], BF, name="p0", tag="pexp")
                            nc.scalar.activation(p0[:], s0[:], AF.Exp)
                            nc.vector.tensor_mul(p0[:, 0:R], p0[:, 0:R],
                                                 msk_sb[:, kb, :])
                            nc.vector.tensor_mul(p0[:, R:2 * R], p0[:, R:2 * R],
                                                 msk_sb[:, kb, :])
                            nc.tensor.matmul(o_ps0[:],
                                             _fr(v_h[:, kb, 0:65]),
                                             _fr(p0[:, 0:R]),
                                             start=(kb == 0), stop=(kb == 15))
                            nc.tensor.matmul(o_ps1[:],
                                             _fr(v_h[:, kb, 65:130]),
                                             _fr(p0[:, R:2 * R]),
                                             start=(kb == 0), stop=(kb == 15))

                        # denominators ride along as psum row 64
                        dt0 = mp.tile([1, R], FT, name="dt0", tag="dt", bufs=4)
                        dt1 = mp.tile([1, R], FT, name="dt1", tag="dt", bufs=4)
                        nc.scalar.copy(dt0[:], o_ps0[64:65, :])
                        nc.scalar.copy(dt1[:], o_ps1[64:65, :])
                        nc.sync.dma_start(den_all[2 * hp:2 * hp + 1, :], dt0[:])
                        nc.sync.dma_start(
                            den_all[2 * hp + 1:2 * hp + 2, :], dt1[:])
                        ot = op.tile([P, R], BF, name=f"o{hp}")
                        nc.vector.tensor_copy(ot[0:64, :], o_ps0[0:64, :])
                        nc.vector.tensor_copy(ot[64:P, :], o_ps1[0:64, :])
                        oT.append(ot)

                    if want('attn'):
                        # one batched reciprocal for all 16 head denominators
                        rden = dsp.tile([16, R], BF, name="rden")
                        nc.vector.reciprocal(rden[:], den_all[:])
                        for hp in range(CT):
                            rdb = pp.tile([P, R], FT, name="rdb", tag="mm")
                            nc.tensor.matmul(rdb[:], sel16_sb[:, hp, :],
                                             rden[:])
                            nc.vector.tensor_mul(oT[hp][:], oT[hp][:], rdb[:])
                    if stage == 'attn':
                        dump(oT)

                    # ================ proj + residual ================
                    for mt in (range(CT) if want('proj') else []):
                        ps = pp.tile([P, R], FT, name="ps_pr", tag="mm")
                        for kt in range(CT):
                            nc.tensor.matmul(ps[:], _fr(wp_all[:, mt, kt, :]),
                                             _fr(oT[kt][:]),
                                             start=(kt == 0), stop=(kt == CT - 1))
                        nc.vector.scalar_tensor_tensor(
                            X[mt][:], ps[:], bproj_sb[:, mt:mt + 1], X[mt][:],
                            op0=OP.add, op1=OP.add)

            # ================== adapters + MLP ==================
            def adapter(d_dw, d_uw, bd_sb, bu_sb, tag):
                with tc.tile_pool(name=f"ad_{tag}", bufs=1) as adp:
                    ad = adp.tile([P, CT, M], BF, name=f"ad_{tag}")
                    nc.sync.dma_start(ad[:], d_dw[:])
                    au = adp.tile([M, CT, P], BF, name=f"au_{tag}")
                    nc.sync.dma_start(au[:], d_uw[:])
                    ps_a = pp.tile([M, R], FT, name=f"psa_{tag}", tag="mm")
                    for kt in range(CT):
                        xbt = adp.tile([P, R], BF, name=f"xb_{tag}", tag="xb",
                                       bufs=3)
                        nc.vector.tensor_copy(xbt[:], X[kt][:])
                        nc.tensor.matmul(ps_a[:], _fr(ad[:, kt, :]),
                                         _fr(xbt[:]),
                                         start=(kt == 0), stop=(kt == CT - 1))
                    ar = adp.tile([M, R], BF, name=f"ar_{tag}")
                    nc.scalar.activation(ar[:], ps_a[:], AF.Relu,
                                         bias=bd_sb[:, 0:1])
                    for mt in range(CT):
                        ps = pp.tile([P, R], FT, name=f"psu_{tag}", tag="mm")
                        nc.tensor.matmul(ps[:], _fr(au[:, mt, :]), _fr(ar[:]))
                        nc.vector.scalar_tensor_tensor(
                            X[mt][:], ps[:], bu_sb[:, mt:mt + 1], X[mt][:],
                            op0=OP.add, op1=OP.add)

            if want('a1'):
                adapter(d_a1d, d_a1u, b1d_sb, b1u_sb, "a1")

            # ---------------- LN2 + MLP ----------------
            with tc.tile_pool(name="xln2", bufs=CT) as x2p, \
                 tc.tile_pool(name="wfcp", bufs=3) as wfp, \
                 tc.tile_pool(name="hT", bufs=32) as hp_, \
                 tc.tile_pool(name="wmpp", bufs=2) as wmp:

                x2 = layer_norm(x2p, ln2g_sb, ln2b_sb, "ln2") if want('mlp') else []
                hT = []
                for mt in (range(32) if want('mlp') else []):
                    wt = wfp.tile([P, CT, P], BF, name="wfc_t", tag="wfc")
                    nc.sync.dma_start(wt[:], d_wfc[mt])
                    ps = pp.tile([P, R], FT, name="ps_fc", tag="mm")
                    for kt in range(CT):
                        nc.tensor.matmul(ps[:], _fr(wt[:, kt, :]), _fr(x2[kt][:]),
                                         start=(kt == 0), stop=(kt == CT - 1))
                    ht = hp_.tile([P, R], BF, name="ht", tag="hT")
                    nc.scalar.activation(ht[:], ps[:], AF.Gelu_apprx_tanh,
                                         bias=bfc_sb[:, mt:mt + 1])
                    hT.append(ht)
                for mt in (range(CT) if want('mlp') else []):
                    wt = wmp.tile([P, 32, P], BF, name="wmp_t", tag="wmp")
                    nc.sync.dma_start(wt[:], d_wmp[mt])
                    ps = pp.tile([P, R], FT, name="ps_mp", tag="mm")
                    for kt in range(32):
                        nc.tensor.matmul(ps[:], _fr(wt[:, kt, :]), _fr(hT[kt][:]),
                                         start=(kt == 0), stop=(kt == 31))
                    nc.vector.scalar_tensor_tensor(
                        X[mt][:], ps[:], bmp_sb[:, mt:mt + 1], X[mt][:],
                        op0=OP.add, op1=OP.add)

            if want('full'):
                adapter(d_a2d, d_a2u, b2d_sb, b2u_sb, "a2")

            # ---------------- output ----------------
            if stage in ('x', 'proj', 'a1', 'mlp', 'full'):
                for ct in range(CT):
                    nc.sync.dma_start(d_out[ct], X[ct][:])

    nc.compile()
    return nc


def _prep_shared(inputs):
    """Host-side tiling of weights/biases into the DRAM layouts above."""
    f32 = np.float32
    bf16 = ml_dtypes.bfloat16
    W = {k: np.ascontiguousarray(np.asarray(v, dtype=f32))
         for k, v in inputs.items()}
    s = f32(1.0 / np.sqrt(HD))
    aw = W['attn_w']
    ab = W['attn_b']
    wq = aw[:, :C] * s
    wk = aw[:, C:2 * C]
    wv = aw[:, 2 * C:]

    def lhst_tiles(w, nmt):
        # w [K, Mout] -> [nmt, P, K//P, P]: tile[mt, p, kt, m] = w[P*kt+p, P*mt+m]
        kk, mm = w.shape
        return np.ascontiguousarray(
            w.reshape(kk // P, P, nmt, P).transpose(2, 1, 0, 3))

    wqk = np.concatenate([lhst_tiles(wq, 8), lhst_tiles(wk, 8)], axis=0)
    # wv moving tiles [2, P, CT, 512]: tile[nt, p, kt, m] = wv[P*kt+p, 512*nt+m]
    wv_m = np.ascontiguousarray(
        wv.reshape(CT, P, 2, 512).transpose(2, 1, 0, 3))

    def col_vec(v, nmt):
        # v [nmt*P] -> [P, nmt]
        return np.ascontiguousarray(v.reshape(nmt, P).T)

    shared = {
        'wqk': wqk,
        'wv': wv_m,
        'wproj': lhst_tiles(W['proj_w'], CT),
        'wfc': lhst_tiles(W['fc_w'], 32),
        'wmp': lhst_tiles(W['mlp_pw'], CT),
        'a1d': np.ascontiguousarray(
            W['a1_dw'].reshape(CT, P, M).transpose(1, 0, 2)),
        'a1u': np.ascontiguousarray(W['a1_uw'].reshape(M, CT, P)),
        'a2d': np.ascontiguousarray(
            W['a2_dw'].reshape(CT, P, M).transpose(1, 0, 2)),
        'a2u': np.ascontiguousarray(W['a2_uw'].reshape(M, CT, P)),
        'bqk': np.ascontiguousarray(
            np.concatenate([ab[:C] * s, ab[C:2 * C]]).reshape(16, P).T),
        'bv': np.ascontiguousarray(ab[2 * C:].reshape(2, 512)),
        'bproj': col_vec(W['proj_b'], CT),
        'bfc': col_vec(W['fc_b'], 32),
        'bmp': col_vec(W['mlp_pb'], CT),
        'b1d': np.ascontiguousarray(W['a1_db'].reshape(M, 1)),
        'b1u': col_vec(W['a1_ub'], CT),
        'b2d': np.ascontiguousarray(W['a2_db'].reshape(M, 1)),
        'b2u': col_vec(W['a2_ub'], CT),
        'ln1g': col_vec(W['ln1_g'], CT),
        'ln1b': col_vec(W['ln1_b'], CT),
        'ln2g': col_vec(W['ln2_g'], CT),
        'ln2b': col_vec(W['ln2_b'], CT),
    }
    selc = np.zeros((2, 5 * P), dtype=f32)
    selc[0, 0:P] = 1.0                       # sel_a: row 0 everywhere
    selc[1, P:2 * P] = 1.0                   # sel_b: row 1 everywhere
    selc[0, 2 * P:2 * P + 64] = 1.0          # sel_h: row m//64
    selc[1, 2 * P + 64:3 * P] = 1.0
    selc[0, 3 * P:3 * P + 64] = 1.0          # sel_lo: partitions 0:64
    selc[0, 4 * P + 64:5 * P] = 1.0          # sel_hi: partitions 64:128
    shared['selc'] = selc
    shared['onesc'] = np.ones((P, 1), dtype=f32)
    shared['onesr'] = np.ones((1, P), dtype=f32)
    shared['vones'] = np.ones((P, 16), dtype=f32)
    sel16 = np.zeros((CT, 16, P), dtype=f32)
    for hp in range(CT):
        sel16[hp, 2 * hp, 0:64] = 1.0
        sel16[hp, 2 * hp + 1, 64:P] = 1.0
    shared['sel16'] = sel16
    for k in ('wqk', 'wv', 'wproj', 'wfc', 'wmp', 'a1d', 'a1u', 'a2d', 'a2u',
              'bv', 'selc', 'onesc', 'onesr', 'vones', 'sel16'):
        shared[k] = np.ascontiguousarray(shared[k].astype(bf16))
    return shared


def _prep_core(x, c):
    # core lc owns tokens {4j + lc} of its batch (stride-4 interleave) so
    # every core has an identical causal block structure
    b, lc = c // 4, c % 4
    xl = np.asarray(x[b, lc::4, :], dtype=np.float32)                 # [R, C]
    xT = np.ascontiguousarray(
        xl.T.reshape(CT, P, R).transpose(1, 0, 2))                    # [P, CT, R]
    # gathered key block kb holds rank (kb//4)'s j-block (kb%4): the key at
    # (partition p, block kb) is global token 4*(128*(kb%4) + p) + kb//4
    p_ = np.arange(P)[:, None, None]
    kb_ = np.arange(16)[None, :, None]
    qi_ = np.arange(R)[None, None, :]
    gk = 4 * (128 * (kb_ % 4) + p_) + kb_ // 4
    gq = 4 * qi_ + lc
    maskT = np.ascontiguousarray(
        (gk <= gq).astype(ml_dtypes.bfloat16))                        # [P,16,R]
    return {'xT': xT, 'mask': maskT}


def _run(inputs, trace=False, stage='full'):
    if stage not in _CACHE:
        _CACHE[stage] = _build(stage)
    nc = _CACHE[stage]
    shared = _prep_shared(inputs)
    x = np.asarray(inputs['x'], dtype=np.float32)
    in_maps = []
    for c in range(NCORES):
        m = dict(shared)
        m.update(_prep_core(x, c))
        in_maps.append(m)
    kwargs = {}
    if trace:
        from trn_agent_boot.trn_boot import _ntff_profile_via_ctypes
        hook = _ntff_profile_via_ctypes('/opt/axon/libaxon_pjrt.so')
        mod = types.ModuleType('antenv.axon_hooks')
        mod.get_axon_ntff_profile_hook = lambda: hook
        sys.modules['antenv.axon_hooks'] = mod
        bass_utils.upload_artifacts = lambda tmpdir: "/tmp/no-upload"
        kwargs['trace'] = True
    res = bass_utils.run_bass_kernel_spmd(
        nc, in_maps, core_ids=list(range(NCORES)), **kwargs)
    y = np.zeros((B, T, C), dtype=np.float32)
    for c in range(NCORES):
        b, lc = c // 4, c % 4
        o = res.results[c]['out']          # [CT, P, R]
        y[b, lc::4, :] = o.reshape(C, R).T
    return y, res


def kernel(**inputs):
    y, _ = _run(inputs, trace=False)
    return y

